# revision 9
# baseline (speedup 1.0000x reference)
"""DeepseekV3 decoder layer on 8 Trainium2 NeuronCores (Bass/Tile).

Sharding: sequence-parallel low-rank projections (one AllGather per latent
group), tensor-parallel heads for q_b/kv_b/attention (2 heads/core,
transposed-score layout), AllGather of head outputs, output-feature-sharded
o_proj + residual, AllGather of the raw post-attention hidden (post-LN stats
are recomputed locally on every core - no stats AllReduce), FF-sharded MLP
with per-chunk ReduceScatter.

Schedule: attention query chunks run in descending size order (3,2,1,0) so
the smallest chunk is last; q_b/rope for chunk j-1 and o_proj/AG3 for chunk
j+1 are interleaved between attention chunks; the MLP consumes AG3 chunks in
arrival order and the final down-proj/ReduceScatter is split 2x256 to shrink
the tail.

RMS scale-invariance: RMS() of a row of (x @ W) is independent of a
per-token scale on x, so the q/kv latent projections run directly on raw
bf16 x with no input RMS on the critical path; only k_pe (64 rows) needs
the 1/rms(x) factor. RMS/ln weights and the rope de-interleave are folded
into the weights host-side.
"""

import numpy as np

B, S, H = 1, 2048, 2048
NH, NOPE, ROPE, VHD = 16, 128, 64, 128
QHD = NOPE + ROPE
QLR, KVLR, FF = 1536, 512, 8192
SCALE = QHD ** -0.5
EPS = 1e-6
NC = 8
SS = S // NC            # 256: sequence / output-feature shard
FFS = FF // NC          # 1024: FF shard
P = 128

TRACE = False           # test.py sets kernel.TRACE = True for profiling

_CACHE = {}


def _tile_w(w):
    """[K, M] -> [K/128, ceil(M/128), 128, 128] contiguous blocks (zero-pad M)."""
    K, M = w.shape
    mc = -(-M // P)
    out = np.zeros((K // P, mc, P, P), np.float32)
    wp = np.zeros((K, mc * P), np.float32)
    wp[:, :M] = w
    for kt in range(K // P):
        for m in range(mc):
            out[kt, m] = wp[kt * P:(kt + 1) * P, m * P:(m + 1) * P]
    return out


def _build():
    if "nc" in _CACHE:
        return _CACHE["nc"]
    import concourse.mybir as mybir
    import concourse.tile as tile
    from concourse import bacc

    F32 = mybir.dt.float32
    F32R = mybir.dt.float32r
    BF16 = mybir.dt.bfloat16
    AF = mybir.ActivationFunctionType

    nc = bacc.Bacc("TRN2", target_bir_lowering=False, debug=False, num_devices=NC)

    def inp(name, shape, dt=F32):
        return nc.dram_tensor(name, list(shape), dt, kind="ExternalInput").ap()

    hT_s = inp("hT_s", [H, SS])
    hT_r = inp("hT_r", [SS, S])
    wq_a_t = inp("wq_a_t", [16, 12, P, P], BF16)
    wkv_a_t = inp("wkv_a_t", [16, 5, P, P], BF16)
    wq_b_t = inp("wq_b_t", [12, 3, P, P], BF16)
    wkv_b_t = inp("wkv_b_t", [4, 4, P, P], BF16)
    wo_t = inp("wo_t", [16, 2, P, P], BF16)
    wg_t = inp("wg_t", [16, 8, P, P], BF16)
    wu_t = inp("wu_t", [16, 8, P, P], BF16)
    wd_t = inp("wd_t", [8, 16, P, P], BF16)
    cossin = inp("cossin", [2 * P, S])        # rows 0:128 [cosT;cosT], 128:256 [sinT;sinT]
    cs_sh = inp("cs_sh", [P, SS])             # rows 0:64 cosT, 64:128 signed sinT (own shard)
    dmask = inp("dmask", [P, 4, 512])
    outT = nc.dram_tensor("outT", [SS, S], F32, kind="ExternalOutput").ap()

    RG = [list(range(NC))]

    from contextlib import ExitStack
    with tile.TileContext(nc) as tc, ExitStack() as _stack:
        cpool = _stack.enter_context(tc.tile_pool(name="const", bufs=1))
        dpool = _stack.enter_context(tc.tile_pool(name="dram", bufs=1, space="DRAM"))
        perm = _stack.enter_context(tc.tile_pool(name="perm", bufs=1))

        ag1a_in = dpool.tile([P, 5 * SS], BF16)
        ag1a_out = dpool.tile([NC * P, 5 * SS], BF16, addr_space="Shared")
        ag1b_in = dpool.tile([P, 12 * SS], BF16)
        ag1b_out = dpool.tile([NC * P, 12 * SS], BF16, addr_space="Shared")
        ag2_in = [dpool.tile([2 * VHD, 1024], BF16, name=f"ag2_in{j}")
                  for j in range(2)]
        ag2_out = [dpool.tile([NH * VHD, 1024], BF16, addr_space="Shared",
                              name=f"ag2_out{j}") for j in range(2)]
        ag3_in = [dpool.tile([SS, 1024], BF16, name=f"ag3_in{j}") for j in range(2)]
        ag3_out = [dpool.tile([H, 1024], BF16, addr_space="Shared",
                              name=f"ag3_out{j}") for j in range(2)]
        # rs chunks: (outT column offset, width), in processing order
        RS_CH = [(1536, 512), (1024, 512), (512, 512), (0, 256), (256, 256)]
        rs_in = [dpool.tile([H, w], BF16, name=f"rs_in{j}")
                 for j, (c0, w) in enumerate(RS_CH)]
        rs_out = [dpool.tile([SS, w], BF16, name=f"rs_out{j}")
                  for j, (c0, w) in enumerate(RS_CH)]

        ones_f = cpool.tile([P, 1], F32)
        nc.vector.memset(ones_f[:], 1.0)
        ones_r = cpool.tile([P, 1], BF16)
        nc.vector.tensor_copy(ones_r[:], ones_f[:])
        eps_t = cpool.tile([P, 1], F32)
        nc.vector.memset(eps_t[:], EPS)
        ones_k1f = cpool.tile([1, P], F32)
        nc.vector.memset(ones_k1f[:], 1.0)
        ones_k1 = cpool.tile([1, P], F32R)
        nc.vector.tensor_copy(ones_k1[:], ones_k1f[:])

        h2 = perm.tile([P, 2, S], F32)        # post-attn hidden, own feature shard

        # shared psum pool for o_proj / stats / down accumulators (phases B+D)
        pbx = _stack.enter_context(tc.tile_pool(name="pbx", bufs=2, space="PSUM"))

        # ================= Stage A: seq-shard low-rank path =================
        with tc.tile_pool(name="sa", bufs=1) as sa, \
             tc.tile_pool(name="saw", bufs=5) as saw, \
             tc.tile_pool(name="pap", bufs=2, space="PSUM") as pa:
            with nc.named_scope("stageA"):
                xs = sa.tile([P, 16, SS], F32)
                nc.sync.dma_start(xs[:], hT_s.rearrange("(kt p) s -> p kt s", p=P))
                xb = sa.tile([P, 16, SS], BF16)
                nc.vector.tensor_copy(xb[:], xs[:])

                # kv latents on raw x (RMS scale-invariance)
                cvs = sa.tile([P, 5, SS], F32)
                for mc in range(5):
                    wt = saw.tile([P, 16, P], BF16, tag="aw")
                    nc.sync.dma_start(wt[:], wkv_a_t[:, mc].rearrange("a p m -> p a m"))
                    ps = pa.tile([P, SS], F32, tag="amm")
                    for kt in range(16):
                        nc.tensor.matmul(ps[:], wt[:, kt], xb[:, kt],
                                         start=(kt == 0), stop=(kt == 15))
                    nc.vector.tensor_copy(cvs[:, mc], ps[:])

                # kv_a RMS (on raw latents; the 1/rms(x) factor cancels)
                sq3 = sa.tile([P, 4, SS], BF16)
                nc.vector.tensor_mul(sq3[:], cvs[:, :4], cvs[:, :4])
                msq3 = pa.tile([1, SS], F32, tag="acc", bufs=1)
                for mc in range(4):
                    nc.tensor.matmul(msq3[:], ones_r[:], sq3[:, mc],
                                     start=(mc == 0), stop=(mc == 3))
                r3s = sa.tile([1, SS], F32)
                nc.scalar.activation(r3s[:], msq3[:], AF.Sqrt, scale=1.0 / KVLR, bias=eps_t[:1])
                r3 = sa.tile([1, SS], F32R)
                with nc.allow_low_precision(reason="f32r rounding of rms scale"):
                    nc.vector.reciprocal(r3[:], r3s[:])
                r3bp = pa.tile([P, SS], F32, tag="rb", bufs=1)
                nc.tensor.matmul(r3bp[:], ones_k1[:], r3[:], start=True, stop=True)
                r3b = sa.tile([P, SS], F32)
                nc.vector.tensor_copy(r3b[:], r3bp[:])
                ckn = sa.tile([P, 4, SS], BF16)
                nc.vector.tensor_mul(ckn[:], cvs[:, :4],
                                     r3b[:, None, :].to_broadcast([P, 4, SS]))

                # rms(x) for the k_pe rows only
                sqx = sa.tile([P, 16, SS], BF16)
                nc.vector.tensor_mul(sqx[:], xb[:], xb[:])
                msq1 = pa.tile([1, SS], F32, tag="acc", bufs=1)
                for kt in range(16):
                    nc.tensor.matmul(msq1[:], ones_r[:], sqx[:, kt],
                                     start=(kt == 0), stop=(kt == 15))
                r1s = sa.tile([1, SS], F32)
                nc.scalar.activation(r1s[:], msq1[:], AF.Sqrt, scale=1.0 / H, bias=eps_t[:1])
                r1 = sa.tile([1, SS], F32R)
                with nc.allow_low_precision(reason="f32r rounding of rms scale"):
                    nc.vector.reciprocal(r1[:], r1s[:])
                r1bp = pa.tile([64, SS], F32, tag="rb", bufs=1)
                nc.tensor.matmul(r1bp[:], ones_k1[:, :64], r1[:], start=True, stop=True)
                r1b = sa.tile([64, SS], F32)
                nc.vector.tensor_copy(r1b[:], r1bp[:])

                # k_pe rope on cvs[:64, 4] (cs_sh rows 0:64 cos, 64:128 signed sin)
                cos_sh = sa.tile([64, SS], F32)
                nc.sync.dma_start(cos_sh[:], cs_sh[0:64, :])
                sin_sh = sa.tile([64, SS], F32)
                nc.sync.dma_start(sin_sh[:], cs_sh[64:128, :])
                ksw = sa.tile([64, SS], F32)
                nc.sync.dma_start(ksw[0:32, :], cvs[32:64, 4])
                nc.sync.dma_start(ksw[32:64, :], cvs[0:32, 4])
                kro = sa.tile([64, SS], F32)
                nc.vector.tensor_mul(kro[:], cvs[:64, 4], cos_sh[:])
                t1 = sa.tile([64, SS], F32)
                nc.vector.tensor_mul(t1[:], ksw[:], sin_sh[:])
                nc.vector.tensor_add(kro[:], kro[:], t1[:])
                kpe_n = sa.tile([64, SS], BF16)
                nc.vector.tensor_mul(kpe_n[:], kro[:], r1b[:])

                nc.sync.dma_start(
                    ag1a_in[:, 0:4 * SS].rearrange("p (kt s) -> p kt s", s=SS),
                    ckn[:])
                nc.sync.dma_start(ag1a_in[:64, 4 * SS:5 * SS], kpe_n[:])
                nc.gpsimd.collective_compute(
                    "AllGather", mybir.AluOpType.bypass, replica_groups=RG,
                    ins=[ag1a_in], outs=[ag1a_out])

                # q latents on raw x
                us = sa.tile([P, 12, SS], F32)
                for mc in range(12):
                    wt = saw.tile([P, 16, P], BF16, tag="aw")
                    nc.sync.dma_start(wt[:], wq_a_t[:, mc].rearrange("a p m -> p a m"))
                    ps = pa.tile([P, SS], F32, tag="amm")
                    for kt in range(16):
                        nc.tensor.matmul(ps[:], wt[:, kt], xb[:, kt],
                                         start=(kt == 0), stop=(kt == 15))
                    nc.vector.tensor_copy(us[:, mc], ps[:])

                sq2 = sa.tile([P, 12, SS], BF16)
                nc.vector.tensor_mul(sq2[:], us[:], us[:])
                msq2 = pa.tile([1, SS], F32, tag="acc", bufs=1)
                for mc in range(12):
                    nc.tensor.matmul(msq2[:], ones_r[:], sq2[:, mc],
                                     start=(mc == 0), stop=(mc == 11))
                r2s = sa.tile([1, SS], F32)
                nc.scalar.activation(r2s[:], msq2[:], AF.Sqrt, scale=1.0 / QLR, bias=eps_t[:1])
                r2 = sa.tile([1, SS], F32R)
                with nc.allow_low_precision(reason="f32r rounding of rms scale"):
                    nc.vector.reciprocal(r2[:], r2s[:])
                r2bp = pa.tile([P, SS], F32, tag="rb", bufs=1)
                nc.tensor.matmul(r2bp[:], ones_k1[:], r2[:], start=True, stop=True)
                r2b = sa.tile([P, SS], F32)
                nc.vector.tensor_copy(r2b[:], r2bp[:])
                un = sa.tile([P, 12, SS], BF16)
                nc.vector.tensor_mul(un[:], us[:],
                                     r2b[:, None, :].to_broadcast([P, 12, SS]))
                nc.sync.dma_start(
                    ag1b_in.rearrange("p (kt s) -> p kt s", s=SS), un[:])
                nc.gpsimd.collective_compute(
                    "AllGather", mybir.AluOpType.bypass, replica_groups=RG,
                    ins=[ag1b_in], outs=[ag1b_out])

        # ===== Stage B: kv_b all blocks, per-chunk q_b/rope + attention =====
        with tc.tile_pool(name="sb2", bufs=1) as sb2, \
             tc.tile_pool(name="sbr", bufs=1) as sbr, \
             tc.tile_pool(name="sbe", bufs=1) as sbe, \
             tc.tile_pool(name="scr", bufs=2) as scr:
            kT = sb2.tile([P, 2, S], BF16)
            kpeT = sb2.tile([64, S], BF16)
            v_tok = sb2.tile([P, 2, 16, P], BF16)
            qT = sb2.tile([P, 2, S], BF16)
            qpe2 = sb2.tile([64, 2, S], BF16)
            oT = sb2.tile([P, 2, S], BF16)
            wkb = sb2.tile([P, 4, 4, P], BF16)
            wqb = sb2.tile([P, 12, 3, P], BF16)
            wos = sb2.tile([P, 16, 2, P], BF16)
            cos_t = sb2.tile([P, S], F32)
            sin_t = sb2.tile([P, S], F32)
            mask_t = sb2.tile([P, 4, 512], F32)
            nc.sync.dma_start(wkb[:], wkv_b_t.rearrange("a b p m -> p a b m"))
            nc.sync.dma_start(wqb[:], wq_b_t.rearrange("a b p m -> p a b m"))
            nc.sync.dma_start(cos_t[:], cossin[0:P, :])
            nc.sync.dma_start(sin_t[:], cossin[P:2 * P, :])
            nc.sync.dma_start(mask_t[:], dmask[:, :, :])
            nc.sync.dma_start(wos[:], wo_t.rearrange("a b p m -> p a b m"))

            def oproj(j):
                """o_proj chunk j + residual -> h2, bf16 h2 into ag3_in half."""
                hf = j // 2
                off = j * 512 - hf * 1024
                nsl = slice(j * 512, (j + 1) * 512)
                rhs = scr.tile([P, 16, 512], BF16, tag="rhs2")
                nc.sync.dma_start(
                    rhs[:],
                    ag2_out[hf].rearrange("(kt p) s -> p kt s", p=P)[
                        :, :, off:off + 512])
                resid = scr.tile([P, 2, 512], F32, tag="resid", bufs=1)
                nc.sync.dma_start(
                    resid[:],
                    hT_r.rearrange("(mc p) s -> p mc s", p=P)[:, :, nsl])
                h2b = scr.tile([P, 2, 512], BF16, tag="h2b")
                for mc in range(2):
                    ps = pbx.tile([P, 512], F32, tag="big")
                    for kt in range(16):
                        nc.tensor.matmul(ps[:], wos[:, kt, mc], rhs[:, kt],
                                         start=(kt == 0), stop=(kt == 15))
                    nc.vector.tensor_add(h2[:, mc, nsl], ps[:], resid[:, mc])
                    nc.vector.tensor_copy(h2b[:, mc], h2[:, mc, nsl])
                nc.sync.dma_start(
                    ag3_in[hf].rearrange("(mc p) s -> p mc s", p=P)[
                        :, :, off:off + 512], h2b[:])

            def ag3_go(hf):
                nc.gpsimd.collective_compute(
                    "AllGather", mybir.AluOpType.bypass, replica_groups=RG,
                    ins=[ag3_in[hf]], outs=[ag3_out[hf]])

            with tc.tile_pool(name="pbq", bufs=2, space="PSUM") as pbq, \
                 tc.tile_pool(name="pbo", bufs=2, space="PSUM") as pbo:
                with nc.named_scope("stageB_kv"):
                    for pr in range(4):
                        psl = slice(pr * 512, (pr + 1) * 512)
                        rhs_c = sbr.tile([P, 4, 2, SS], BF16, tag="rhs1c", bufs=2)
                        for b in range(2):
                            blk = 2 * pr + b
                            nc.sync.dma_start(
                                rhs_c[:, :, b, :],
                                ag1a_out[blk * P:(blk + 1) * P, 0:4 * SS].rearrange(
                                    "p (kt s) -> p kt s", s=SS))
                            nc.sync.dma_start(
                                kpeT[:, blk * SS:(blk + 1) * SS],
                                ag1a_out[blk * P:blk * P + 64, 4 * SS:5 * SS])
                        # k_nope (dim-major)
                        for mc in range(2):
                            ps = pbq.tile([P, 512], F32, tag="sc")
                            for kt in range(4):
                                nc.tensor.matmul(
                                    ps[:], wkb[:, kt, mc],
                                    rhs_c[:, kt].rearrange("p b s -> p (b s)"),
                                    start=(kt == 0), stop=(kt == 3))
                            nc.vector.tensor_copy(kT[:, mc, psl], ps[:])
                        # V token-major: stationary = latent tile, moving = v-cols
                        for b in range(2):
                            for st2 in range(2):
                                stile = pr * 4 + b * 2 + st2
                                pv = pbo.tile([P, 2, P], F32, tag="o")
                                for kt in range(4):
                                    nc.tensor.matmul(
                                        pv[:].rearrange("p h v -> p (h v)"),
                                        rhs_c[:, kt, b, st2 * P:(st2 + 1) * P],
                                        wkb[:, kt, 2:4, :].rearrange("p h v -> p (h v)"),
                                        start=(kt == 0), stop=(kt == 3))
                                nc.vector.tensor_copy(v_tok[:, :, stile, :], pv[:])

                def qb_rope(pr):
                    """q_b + rope for token pair-block pr (512 tokens)."""
                    psl = slice(pr * 512, (pr + 1) * 512)
                    rhs_u = sbr.tile([P, 12, 2, SS], BF16, tag="rhs1u", bufs=2)
                    for b in range(2):
                        blk = 2 * pr + b
                        nc.sync.dma_start(
                            rhs_u[:, :, b, :],
                            ag1b_out[blk * P:(blk + 1) * P, :].rearrange(
                                "p (kt s) -> p kt s", s=SS))
                    qpe_raw = sbr.tile([P, 512], F32, tag="qpr")
                    for mc in range(3):
                        ps = pbq.tile([P, 512], F32, tag="sc")
                        for kt in range(12):
                            nc.tensor.matmul(
                                ps[:], wqb[:, kt, mc],
                                rhs_u[:, kt].rearrange("p b s -> p (b s)"),
                                start=(kt == 0), stop=(kt == 11))
                        if mc < 2:
                            nc.vector.tensor_copy(qT[:, mc, psl], ps[:])
                        else:
                            nc.vector.tensor_copy(qpe_raw[:], ps[:])
                    qsw = sbr.tile([P, 512], F32, tag="qsw")
                    for qq in range(2):
                        b0 = qq * 64
                        nc.sync.dma_start(qsw[b0:b0 + 32, :],
                                          qpe_raw[b0 + 32:b0 + 64, :])
                        nc.sync.dma_start(qsw[b0 + 32:b0 + 64, :],
                                          qpe_raw[b0:b0 + 32, :])
                    qpe_rot = sbr.tile([P, 512], BF16, tag="qro")
                    nc.vector.tensor_mul(qpe_rot[:], qpe_raw[:], cos_t[:, psl])
                    t1r = sbr.tile([P, 512], F32, tag="qt1")
                    nc.vector.tensor_mul(t1r[:], qsw[:], sin_t[:, psl])
                    nc.vector.tensor_add(qpe_rot[:], qpe_rot[:], t1r[:])
                    nc.sync.dma_start(qpe2[:, 0, psl], qpe_rot[0:64, :])
                    nc.sync.dma_start(qpe2[:, 1, psl], qpe_rot[64:128, :])

                def attn_chunk(qc):
                    qsl = slice(qc * 512, (qc + 1) * 512)
                    nkt = 4 * qc + 4
                    for h in range(2):
                        o_ps = pbo.tile([P, 512], F32, tag="o")
                        d_ps = pbx.tile([1, 512], F32, tag="acc")
                        for kt in range(nkt):
                            ksl = slice(kt * P, (kt + 1) * P)
                            sc_ps = pbq.tile([P, 512], F32, tag="sc")
                            nc.tensor.matmul(sc_ps[:], kT[:, h, ksl],
                                             qT[:, h, qsl], start=True, stop=False)
                            nc.tensor.matmul(sc_ps[:], kpeT[:, ksl],
                                             qpe2[:, h, qsl], start=False, stop=True)
                            j = kt - 4 * qc
                            if j >= 0:
                                nc.vector.tensor_add(sc_ps[:], sc_ps[:],
                                                     mask_t[:, j])
                            es = sbe.tile([P, 512], BF16, tag="es", bufs=4)
                            nc.scalar.activation(es[:], sc_ps[:], AF.Exp)
                            nc.tensor.matmul(o_ps[:], v_tok[:, h, kt], es[:],
                                             start=(kt == 0), stop=(kt == nkt - 1))
                            nc.tensor.matmul(d_ps[:], ones_r[:], es[:],
                                             start=(kt == 0), stop=(kt == nkt - 1))
                        rec = sbe.tile([1, 512], F32R, tag="rec", bufs=2)
                        with nc.allow_low_precision(
                                reason="f32r rounding of softmax denom"):
                            nc.vector.reciprocal(rec[:], d_ps[:])
                        rb_ps = pbx.tile([P, 512], F32, tag="big")
                        nc.tensor.matmul(rb_ps[:], ones_k1[:], rec[:],
                                         start=True, stop=True)
                        recb = sbe.tile([P, 512], F32, tag="recb", bufs=2)
                        nc.vector.tensor_copy(recb[:], rb_ps[:])
                        nc.vector.tensor_mul(oT[:, h, qsl], o_ps[:], recb[:])

                def ag2_go(hf):
                    nc.sync.dma_start(
                        ag2_in[hf].rearrange("(mc p) s -> p mc s", p=P),
                        oT[:, :, hf * 1024:(hf + 1) * 1024])
                    nc.gpsimd.collective_compute(
                        "AllGather", mybir.AluOpType.bypass, replica_groups=RG,
                        ins=[ag2_in[hf]], outs=[ag2_out[hf]])

                with nc.named_scope("stageB_attn"):
                    qb_rope(3)
                    qb_rope(2)
                    attn_chunk(3)
                    qb_rope(1)
                    attn_chunk(2)
                    ag2_go(1)
                    qb_rope(0)
                    attn_chunk(1)
                    oproj(3)
                    oproj(2)
                    ag3_go(1)
                    attn_chunk(0)
                    ag2_go(0)
                    oproj(1)
                    oproj(0)
                    ag3_go(0)

        # ================= Stage D: post-LN + MLP, chunk pipelined =============
        with tc.tile_pool(name="wmlp", bufs=1) as wmlp, \
             tc.tile_pool(name="wstr", bufs=1) as wstr, \
             tc.tile_pool(name="smy", bufs=1) as smy, \
             tc.tile_pool(name="sdd", bufs=2) as sdd, \
             tc.tile_pool(name="pgu", bufs=1, space="PSUM") as pgu:
            with nc.named_scope("stageD"):
                wds = wmlp.tile([P, 8, 16, P], BF16)
                nc.sync.dma_start(wds[:], wd_t.rearrange("a b p m -> p a b m"))

                def hy_fetch(j):
                    hf = j // 2
                    off = j * 512 - hf * 1024
                    hy = smy.tile([P, 16, 512], BF16, tag="hy", bufs=3)
                    nc.sync.dma_start(
                        hy[:],
                        ag3_out[hf].rearrange("(kt p) s -> p kt s", p=P)[
                            :, :, off:off + 512])
                    return hy

                def stats(hy):
                    """Local post-LN stats; scales hy into y in place."""
                    m4 = pbx.tile([1, 512], F32, tag="acc")
                    for half in range(2):
                        sqh = smy.tile([P, 8, 512], BF16, tag="sqh", bufs=1)
                        nc.vector.tensor_mul(sqh[:], hy[:, half * 8:half * 8 + 8],
                                             hy[:, half * 8:half * 8 + 8])
                        for kt in range(8):
                            nc.tensor.matmul(m4[:], ones_r[:], sqh[:, kt],
                                             start=(half == 0 and kt == 0),
                                             stop=(half == 1 and kt == 7))
                    r4s = smy.tile([1, 512], F32, tag="r4s", bufs=2)
                    nc.scalar.activation(r4s[:], m4[:], AF.Sqrt,
                                         scale=1.0 / H, bias=eps_t[:1])
                    r4 = smy.tile([1, 512], F32R, tag="r4", bufs=2)
                    with nc.allow_low_precision(reason="f32r rounding of rms scale"):
                        nc.vector.reciprocal(r4[:], r4s[:])
                    r4bp = pbx.tile([P, 512], F32, tag="big")
                    nc.tensor.matmul(r4bp[:], ones_k1[:], r4[:],
                                     start=True, stop=True)
                    r4b = smy.tile([P, 512], F32, tag="r4b", bufs=2)
                    nc.vector.tensor_copy(r4b[:], r4bp[:])
                    nc.vector.tensor_mul(hy[:], hy[:],
                                         r4b[:, None, :].to_broadcast([P, 16, 512]))
                    return hy

                def gateup(y, y_off, cw):
                    """gate/up on y[:, :, y_off:y_off+cw], streaming weights."""
                    act = smy.tile([P, 8, 512], BF16, tag="act", bufs=2,
                                   name="act")[:, :, :cw]
                    for m in range(8):
                        wgm = wstr.tile([P, 16, P], BF16, tag="wg", bufs=4)
                        nc.sync.dma_start(
                            wgm[:], wg_t[:, m].rearrange("a p m -> p a m"))
                        wum = wstr.tile([P, 16, P], BF16, tag="wu", bufs=4)
                        nc.sync.dma_start(
                            wum[:], wu_t[:, m].rearrange("a p m -> p a m"))
                        gp = pgu.tile([P, 512], F32, tag=f"g{m % 2}",
                                      name="gp")[:, :cw]
                        up = pgu.tile([P, 512], F32, tag=f"u{m % 2}",
                                      name="up")[:, :cw]
                        for kt in range(16):
                            nc.tensor.matmul(gp[:], wgm[:, kt],
                                             y[:, kt, y_off:y_off + cw],
                                             start=(kt == 0), stop=(kt == 15))
                            nc.tensor.matmul(up[:], wum[:, kt],
                                             y[:, kt, y_off:y_off + cw],
                                             start=(kt == 0), stop=(kt == 15))
                        gsil = sdd.tile([P, 512], BF16, tag="gsil",
                                        name="gsil")[:, :cw]
                        nc.scalar.activation(gsil[:], gp[:], AF.Silu)
                        nc.vector.tensor_mul(act[:, m], gsil[:], up[:])
                    return act

                def down(act, act_off, ri):
                    """down-proj of act[:, :, act_off:act_off+w] -> RS chunk ri."""
                    c0, cw = RS_CH[ri]
                    nsl = slice(c0, c0 + cw)
                    for mc in range(16):
                        ps = pbx.tile([P, 512], F32, tag="big", name="dps")[:, :cw]
                        for kt in range(8):
                            nc.tensor.matmul(
                                ps[:], wds[:, kt, mc],
                                act[:, kt, act_off:act_off + cw],
                                start=(kt == 0), stop=(kt == 7))
                        dn = sdd.tile([P, 512], BF16, tag="dn", name="dn")[:, :cw]
                        if mc % 2 == 0:
                            nc.vector.tensor_copy(dn[:], ps[:])
                        else:
                            nc.scalar.activation(dn[:], ps[:], AF.Copy)
                        nc.sync.dma_start(rs_in[ri][mc * P:(mc + 1) * P, :], dn[:])
                    nc.gpsimd.collective_compute(
                        "ReduceScatter", mybir.AluOpType.add, replica_groups=RG,
                        ins=[rs_in[ri]], outs=[rs_out[ri]])
                    fin = sdd.tile([P, 2, 512], BF16, tag="fin",
                                   name="fin")[:, :, :cw]
                    nc.sync.dma_start(
                        fin[:], rs_out[ri].rearrange("(mc p) s -> p mc s", p=P))
                    fino = sdd.tile([P, 2, 512], F32, tag="fino", name="fino",
                                    bufs=1)[:, :, :cw]
                    nc.vector.tensor_add(fino[:], fin[:], h2[:, :, nsl])
                    nc.sync.dma_start(
                        outT.rearrange("(mc p) s -> p mc s", p=P)[:, :, nsl],
                        fino[:])

                hy3 = hy_fetch(3)
                y3 = stats(hy3)
                hy2 = hy_fetch(2)
                act3 = gateup(y3, 0, 512)
                y2 = stats(hy2)
                down(act3, 0, 0)
                hy1 = hy_fetch(1)
                act2 = gateup(y2, 0, 512)
                y1 = stats(hy1)
                down(act2, 0, 1)
                hy0 = hy_fetch(0)
                act1 = gateup(y1, 0, 512)
                y0 = stats(hy0)
                down(act1, 0, 2)
                act0a = gateup(y0, 0, 256)
                down(act0a, 0, 3)
                act0b = gateup(y0, 256, 256)
                down(act0b, 0, 4)

    nc.compile()
    _CACHE["nc"] = nc
    return nc


def _host_prep(inputs):
    import ml_dtypes
    bf16 = ml_dtypes.bfloat16
    inp = {k: np.asarray(v) for k, v in inputs.items()}
    hidden = inp["hidden_states"].reshape(S, H).astype(np.float32)
    pos = inp["position_ids"].reshape(S).astype(np.int64)
    cosT = inp["cos"][pos].T.astype(np.float32)
    sinT = inp["sin"][pos].T.astype(np.float32)
    wq_a = (inp["wq_a"] * inp["in_ln"][:, None]).astype(np.float32)
    wkv_a = (inp["wkv_a"] * inp["in_ln"][:, None]).astype(np.float32)
    wq_b = (inp["wq_b"] * inp["q_a_ln"][:, None]).astype(np.float32)
    wkv_b = (inp["wkv_b"] * inp["kv_a_ln"][:, None]).astype(np.float32)
    wg = (inp["w_gate"] * inp["post_ln"][:, None]).astype(np.float32)
    wu = (inp["w_up"] * inp["post_ln"][:, None]).astype(np.float32)
    wd = inp["w_down"].astype(np.float32)
    wo = inp["wo"].astype(np.float32)

    de = np.empty(ROPE, np.int64)
    de[:32] = np.arange(32) * 2
    de[32:] = np.arange(32) * 2 + 1
    wkv_a = np.concatenate([wkv_a[:, :KVLR], wkv_a[:, KVLR:][:, de]], axis=1)
    wq_b = wq_b.reshape(QLR, NH, QHD)
    wkv_b = wkv_b.reshape(KVLR, NH, NOPE + VHD)

    hT = hidden.T.copy()
    sin_sg = np.concatenate([-sinT[:32], sinT[32:]], axis=0)    # signed for swap trick
    cossin = np.concatenate([cosT, cosT, sin_sg, sin_sg], axis=0)  # (256, S)
    ki = np.arange(P)[:, None]
    qi = np.arange(512)[None, :]
    dmask = np.stack([np.where(qi >= j * P + ki, 0.0, -1e30).astype(np.float32)
                      for j in range(4)], axis=1)               # (128, 4, 512)

    wq_a_t = _tile_w(wq_a)
    wkv_a_t = _tile_w(wkv_a)

    in_maps = []
    for c in range(NC):
        h0, h1 = 2 * c, 2 * c + 1
        qb = np.concatenate([
            wq_b[:, h0, :NOPE], wq_b[:, h1, :NOPE],
            wq_b[:, h0, NOPE:][:, de], wq_b[:, h1, NOPE:][:, de]], axis=1) * SCALE
        kb = np.concatenate([
            wkv_b[:, h0, :NOPE], wkv_b[:, h1, :NOPE],
            wkv_b[:, h0, NOPE:], wkv_b[:, h1, NOPE:]], axis=1)
        ssl = slice(c * SS, (c + 1) * SS)
        cs_sh = np.concatenate([cosT[:, ssl], sin_sg[:, ssl]], axis=0)
        in_maps.append({
            "hT_s": np.ascontiguousarray(hT[:, ssl]),
            "hT_r": np.ascontiguousarray(hT[ssl, :]),
            "wq_a_t": wq_a_t.astype(bf16),
            "wkv_a_t": wkv_a_t.astype(bf16),
            "wq_b_t": _tile_w(qb.astype(np.float32)).astype(bf16),
            "wkv_b_t": _tile_w(kb.astype(np.float32)).astype(bf16),
            "wo_t": _tile_w(np.ascontiguousarray(wo[:, ssl])).astype(bf16),
            "wg_t": _tile_w(wg[:, c * FFS:(c + 1) * FFS]).astype(bf16),
            "wu_t": _tile_w(wu[:, c * FFS:(c + 1) * FFS]).astype(bf16),
            "wd_t": _tile_w(wd[c * FFS:(c + 1) * FFS, :]).astype(bf16),
            "cossin": cossin,
            "cs_sh": np.ascontiguousarray(cs_sh),
            "dmask": dmask,
        })
    return in_maps


_LAST_RESULT = {}


def kernel(**inputs) -> np.ndarray:
    from concourse.bass_utils import run_bass_kernel_spmd
    nc = _build()
    in_maps = _host_prep(inputs)
    kwargs = {}
    if TRACE:
        import sys, types
        if "antenv.axon_hooks" not in sys.modules:
            try:
                from trn_agent_boot.trn_boot import _ntff_profile_via_ctypes
                mod = types.ModuleType("antenv.axon_hooks")
                _hook = _ntff_profile_via_ctypes('/opt/axon/libaxon_pjrt.so')
                mod.get_axon_ntff_profile_hook = lambda: _hook
                mod.set_axon_ntff_profile_hook = lambda h: None
                sys.modules["antenv.axon_hooks"] = mod
                import antenv
                antenv.axon_hooks = mod
            except Exception:
                pass
        kwargs["trace"] = True
    res = run_bass_kernel_spmd(nc, in_maps, list(range(NC)), **kwargs)
    _LAST_RESULT["res"] = res
    outT = np.concatenate([res.results[c]["outT"] for c in range(NC)], axis=0)
    return np.ascontiguousarray(outT.T)[None].astype(np.float32)


# revision 11
# speedup vs baseline: 1.0870x; 1.0870x over previous
"""DeepseekV3 decoder layer on 8 Trainium2 NeuronCores (Bass/Tile).

Sharding: sequence-parallel low-rank projections (one AllGather per latent
group), tensor-parallel heads for q_b/kv_b/attention (2 heads/core,
transposed-score layout), AllGather of head outputs, output-feature-sharded
o_proj + residual, AllGather of the raw post-attention hidden (post-LN stats
are recomputed locally on every core - no stats AllReduce), FF-sharded MLP
with per-chunk ReduceScatter.

Schedule: attention query chunks run in descending size order (3,2,1,0) so
the smallest chunk is last; q_b/rope for chunk j-1 and o_proj/AG3 for chunk
j+1 are interleaved between attention chunks; the MLP consumes AG3 chunks in
arrival order and the final down-proj/ReduceScatter is split 2x256 to shrink
the tail.

RMS scale-invariance: RMS() of a row of (x @ W) is independent of a
per-token scale on x, so the q/kv latent projections run directly on raw
bf16 x with no input RMS on the critical path; only k_pe (64 rows) needs
the 1/rms(x) factor. RMS/ln weights and the rope de-interleave are folded
into the weights host-side.
"""

import numpy as np

B, S, H = 1, 2048, 2048
NH, NOPE, ROPE, VHD = 16, 128, 64, 128
QHD = NOPE + ROPE
QLR, KVLR, FF = 1536, 512, 8192
SCALE = QHD ** -0.5
EPS = 1e-6
NC = 8
SS = S // NC            # 256: sequence / output-feature shard
FFS = FF // NC          # 1024: FF shard
P = 128

TRACE = False           # test.py sets kernel.TRACE = True for profiling

_CACHE = {}


def _tile_w(w):
    """[K, M] -> [K/128, ceil(M/128), 128, 128] contiguous blocks (zero-pad M)."""
    K, M = w.shape
    mc = -(-M // P)
    out = np.zeros((K // P, mc, P, P), np.float32)
    wp = np.zeros((K, mc * P), np.float32)
    wp[:, :M] = w
    for kt in range(K // P):
        for m in range(mc):
            out[kt, m] = wp[kt * P:(kt + 1) * P, m * P:(m + 1) * P]
    return out


def _build():
    if "nc" in _CACHE:
        return _CACHE["nc"]
    import concourse.mybir as mybir
    import concourse.tile as tile
    from concourse import bacc

    F32 = mybir.dt.float32
    F32R = mybir.dt.float32r
    BF16 = mybir.dt.bfloat16
    AF = mybir.ActivationFunctionType

    nc = bacc.Bacc("TRN2", target_bir_lowering=False, debug=False, num_devices=NC)

    def inp(name, shape, dt=F32):
        return nc.dram_tensor(name, list(shape), dt, kind="ExternalInput").ap()

    hT_s = inp("hT_s", [H, SS])
    hT_r = inp("hT_r", [SS, S])
    wq_a_t = inp("wq_a_t", [12, P, 16, P], BF16)
    wkv_a_t = inp("wkv_a_t", [5, P, 16, P], BF16)
    wq_b_t = inp("wq_b_t", [P, 12, 3, P], BF16)
    wkv_b_t = inp("wkv_b_t", [P, 4, 4, P], BF16)
    wo_t = inp("wo_t", [P, 16, 2, P], BF16)
    wg_t = inp("wg_t", [8, P, 16, P], BF16)
    wu_t = inp("wu_t", [8, P, 16, P], BF16)
    wd_t = inp("wd_t", [P, 8, 16, P], BF16)
    cossin = inp("cossin", [2 * P, S])        # rows 0:128 [cosT;cosT], 128:256 [sinT;sinT]
    cs_sh = inp("cs_sh", [P, SS])             # rows 0:64 cosT, 64:128 signed sinT (own shard)
    dmask = inp("dmask", [P, 4, 512])
    outT = nc.dram_tensor("outT", [SS, S], F32, kind="ExternalOutput").ap()

    RG = [list(range(NC))]

    from contextlib import ExitStack
    with tile.TileContext(nc) as tc, ExitStack() as _stack:
        cpool = _stack.enter_context(tc.tile_pool(name="const", bufs=1))
        dpool = _stack.enter_context(tc.tile_pool(name="dram", bufs=1, space="DRAM"))
        perm = _stack.enter_context(tc.tile_pool(name="perm", bufs=1))

        ag1a_in = dpool.tile([P, 5 * SS], BF16)
        ag1a_out = dpool.tile([NC * P, 5 * SS], BF16, addr_space="Shared")
        ag1b_in = dpool.tile([P, 12 * SS], BF16)
        ag1b_out = dpool.tile([NC * P, 12 * SS], BF16, addr_space="Shared")
        _ag2w = [1024, 512, 512]          # h1 (chunks 3+2), c1, c0
        ag2_in = [dpool.tile([2 * VHD, w], BF16, name=f"ag2_in{j}")
                  for j, w in enumerate(_ag2w)]
        ag2_out = [dpool.tile([NH * VHD, w], BF16, addr_space="Shared",
                              name=f"ag2_out{j}") for j, w in enumerate(_ag2w)]
        # per attn chunk j: (ag2 buffer index, col offset, oT col offset)
        AG2_MAP = {3: (0, 0, 1024), 2: (0, 512, 1024), 1: (1, 0, 512),
                   0: (2, 0, 0)}
        ag3_in = [dpool.tile([SS, 1024], BF16, name=f"ag3_in{j}") for j in range(2)]
        ag3_out = [dpool.tile([H, 1024], BF16, addr_space="Shared",
                              name=f"ag3_out{j}") for j in range(2)]
        # rs chunks: (outT column offset, width), in processing order
        RS_CH = [(1536, 512), (1024, 512), (512, 512), (0, 256), (256, 256)]
        rs_in = [dpool.tile([H, w], BF16, name=f"rs_in{j}")
                 for j, (c0, w) in enumerate(RS_CH)]
        rs_out = [dpool.tile([SS, w], BF16, name=f"rs_out{j}")
                  for j, (c0, w) in enumerate(RS_CH)]

        ones_f = cpool.tile([P, 1], F32)
        nc.vector.memset(ones_f[:], 1.0)
        ones_r = cpool.tile([P, 1], BF16)
        nc.vector.tensor_copy(ones_r[:], ones_f[:])
        eps_t = cpool.tile([P, 1], F32)
        nc.vector.memset(eps_t[:], EPS)
        ones_k1f = cpool.tile([1, P], F32)
        nc.vector.memset(ones_k1f[:], 1.0)
        ones_k1 = cpool.tile([1, P], F32R)
        nc.vector.tensor_copy(ones_k1[:], ones_k1f[:])

        h2 = perm.tile([P, 2, S], F32)        # post-attn hidden, own feature shard

        # shared psum pool for o_proj / stats / down accumulators (phases B+D)
        pbx = _stack.enter_context(tc.tile_pool(name="pbx", bufs=2, space="PSUM"))

        # ================= Stage A: seq-shard low-rank path =================
        with tc.tile_pool(name="sa", bufs=1) as sa, \
             tc.tile_pool(name="saw", bufs=5) as saw, \
             tc.tile_pool(name="pap", bufs=2, space="PSUM") as pa:
            with nc.named_scope("stageA"):
                xs = sa.tile([P, 16, SS], F32)
                nc.sync.dma_start(xs[:], hT_s.rearrange("(kt p) s -> p kt s", p=P))
                xb = sa.tile([P, 16, SS], BF16)
                nc.vector.tensor_copy(xb[:], xs[:])

                # kv latents on raw x (RMS scale-invariance)
                cvs = sa.tile([P, 5, SS], F32)
                for mc in range(5):
                    wt = saw.tile([P, 16, P], BF16, tag="aw")
                    nc.sync.dma_start(wt[:], wkv_a_t[mc])
                    ps = pa.tile([P, SS], F32, tag="amm")
                    for kt in range(16):
                        nc.tensor.matmul(ps[:], wt[:, kt], xb[:, kt],
                                         start=(kt == 0), stop=(kt == 15))
                    nc.vector.tensor_copy(cvs[:, mc], ps[:])

                # kv_a RMS (on raw latents; the 1/rms(x) factor cancels)
                sq3 = sa.tile([P, 4, SS], BF16)
                nc.vector.tensor_mul(sq3[:], cvs[:, :4], cvs[:, :4])
                msq3 = pa.tile([1, SS], F32, tag="acc", bufs=1)
                for mc in range(4):
                    nc.tensor.matmul(msq3[:], ones_r[:], sq3[:, mc],
                                     start=(mc == 0), stop=(mc == 3))
                r3s = sa.tile([1, SS], F32)
                nc.scalar.activation(r3s[:], msq3[:], AF.Sqrt, scale=1.0 / KVLR, bias=eps_t[:1])
                r3 = sa.tile([1, SS], F32R)
                with nc.allow_low_precision(reason="f32r rounding of rms scale"):
                    nc.vector.reciprocal(r3[:], r3s[:])
                r3bp = pa.tile([P, SS], F32, tag="rb", bufs=1)
                nc.tensor.matmul(r3bp[:], ones_k1[:], r3[:], start=True, stop=True)
                r3b = sa.tile([P, SS], F32)
                nc.vector.tensor_copy(r3b[:], r3bp[:])
                ckn = sa.tile([P, 4, SS], BF16)
                nc.vector.tensor_mul(ckn[:], cvs[:, :4],
                                     r3b[:, None, :].to_broadcast([P, 4, SS]))

                # rms(x) for the k_pe rows only
                sqx = sa.tile([P, 16, SS], BF16)
                nc.vector.tensor_mul(sqx[:], xb[:], xb[:])
                msq1 = pa.tile([1, SS], F32, tag="acc", bufs=1)
                for kt in range(16):
                    nc.tensor.matmul(msq1[:], ones_r[:], sqx[:, kt],
                                     start=(kt == 0), stop=(kt == 15))
                r1s = sa.tile([1, SS], F32)
                nc.scalar.activation(r1s[:], msq1[:], AF.Sqrt, scale=1.0 / H, bias=eps_t[:1])
                r1 = sa.tile([1, SS], F32R)
                with nc.allow_low_precision(reason="f32r rounding of rms scale"):
                    nc.vector.reciprocal(r1[:], r1s[:])
                r1bp = pa.tile([64, SS], F32, tag="rb", bufs=1)
                nc.tensor.matmul(r1bp[:], ones_k1[:, :64], r1[:], start=True, stop=True)
                r1b = sa.tile([64, SS], F32)
                nc.vector.tensor_copy(r1b[:], r1bp[:])

                # k_pe rope on cvs[:64, 4] (cs_sh rows 0:64 cos, 64:128 signed sin)
                cos_sh = sa.tile([64, SS], F32)
                nc.sync.dma_start(cos_sh[:], cs_sh[0:64, :])
                sin_sh = sa.tile([64, SS], F32)
                nc.sync.dma_start(sin_sh[:], cs_sh[64:128, :])
                ksw = sa.tile([64, SS], F32)
                nc.sync.dma_start(ksw[0:32, :], cvs[32:64, 4])
                nc.sync.dma_start(ksw[32:64, :], cvs[0:32, 4])
                kro = sa.tile([64, SS], F32)
                nc.vector.tensor_mul(kro[:], cvs[:64, 4], cos_sh[:])
                t1 = sa.tile([64, SS], F32)
                nc.vector.tensor_mul(t1[:], ksw[:], sin_sh[:])
                nc.vector.tensor_add(kro[:], kro[:], t1[:])
                kpe_n = sa.tile([64, SS], BF16)
                nc.vector.tensor_mul(kpe_n[:], kro[:], r1b[:])

                nc.sync.dma_start(
                    ag1a_in[:, 0:4 * SS].rearrange("p (kt s) -> p kt s", s=SS),
                    ckn[:])
                nc.sync.dma_start(ag1a_in[:64, 4 * SS:5 * SS], kpe_n[:])
                nc.gpsimd.collective_compute(
                    "AllGather", mybir.AluOpType.bypass, replica_groups=RG,
                    ins=[ag1a_in], outs=[ag1a_out])

                # q latents on raw x
                us = sa.tile([P, 12, SS], F32)
                for mc in range(12):
                    wt = saw.tile([P, 16, P], BF16, tag="aw")
                    nc.sync.dma_start(wt[:], wq_a_t[mc])
                    ps = pa.tile([P, SS], F32, tag="amm")
                    for kt in range(16):
                        nc.tensor.matmul(ps[:], wt[:, kt], xb[:, kt],
                                         start=(kt == 0), stop=(kt == 15))
                    nc.vector.tensor_copy(us[:, mc], ps[:])

                sq2 = sa.tile([P, 12, SS], BF16)
                nc.vector.tensor_mul(sq2[:], us[:], us[:])
                msq2 = pa.tile([1, SS], F32, tag="acc", bufs=1)
                for mc in range(12):
                    nc.tensor.matmul(msq2[:], ones_r[:], sq2[:, mc],
                                     start=(mc == 0), stop=(mc == 11))
                r2s = sa.tile([1, SS], F32)
                nc.scalar.activation(r2s[:], msq2[:], AF.Sqrt, scale=1.0 / QLR, bias=eps_t[:1])
                r2 = sa.tile([1, SS], F32R)
                with nc.allow_low_precision(reason="f32r rounding of rms scale"):
                    nc.vector.reciprocal(r2[:], r2s[:])
                r2bp = pa.tile([P, SS], F32, tag="rb", bufs=1)
                nc.tensor.matmul(r2bp[:], ones_k1[:], r2[:], start=True, stop=True)
                r2b = sa.tile([P, SS], F32)
                nc.vector.tensor_copy(r2b[:], r2bp[:])
                un = sa.tile([P, 12, SS], BF16)
                nc.vector.tensor_mul(un[:], us[:],
                                     r2b[:, None, :].to_broadcast([P, 12, SS]))
                nc.sync.dma_start(
                    ag1b_in.rearrange("p (kt s) -> p kt s", s=SS), un[:])
                nc.gpsimd.collective_compute(
                    "AllGather", mybir.AluOpType.bypass, replica_groups=RG,
                    ins=[ag1b_in], outs=[ag1b_out])

        # ===== Stage B: kv_b all blocks, per-chunk q_b/rope + attention =====
        with tc.tile_pool(name="sb2", bufs=1) as sb2, \
             tc.tile_pool(name="sbr", bufs=1) as sbr, \
             tc.tile_pool(name="sbe", bufs=1) as sbe, \
             tc.tile_pool(name="scr", bufs=2) as scr:
            kT = sb2.tile([P, 2, S], BF16)
            kpeT = sb2.tile([64, S], BF16)
            v_tok = sb2.tile([P, 2, 16, P], BF16)
            qT = sb2.tile([P, 2, S], BF16)
            qpe2 = sb2.tile([64, 2, S], BF16)
            oT = sb2.tile([P, 2, S], BF16)
            wkb = sb2.tile([P, 4, 4, P], BF16)
            wqb = sb2.tile([P, 12, 3, P], BF16)
            wos = sb2.tile([P, 16, 2, P], BF16)
            cos_t = sb2.tile([P, S], F32)
            sin_t = sb2.tile([P, S], F32)
            mask_t = sb2.tile([P, 4, 512], F32)
            nc.sync.dma_start(wkb[:], wkv_b_t[:])
            nc.sync.dma_start(wqb[:], wq_b_t[:])
            nc.sync.dma_start(cos_t[:], cossin[0:P, :])
            nc.sync.dma_start(sin_t[:], cossin[P:2 * P, :])
            nc.sync.dma_start(mask_t[:], dmask[:, :, :])
            nc.sync.dma_start(wos[:], wo_t[:])

            def oproj(j):
                """o_proj chunk j + residual -> h2, bf16 h2 into ag3_in half."""
                hf = j // 2
                off = j * 512 - hf * 1024
                bi, boff, _ = AG2_MAP[j]
                nsl = slice(j * 512, (j + 1) * 512)
                rhs = scr.tile([P, 16, 512], BF16, tag="rhs2")
                nc.sync.dma_start(
                    rhs[:],
                    ag2_out[bi].rearrange("(kt p) s -> p kt s", p=P)[
                        :, :, boff:boff + 512])
                resid = scr.tile([P, 2, 512], F32, tag="resid", bufs=1)
                nc.sync.dma_start(
                    resid[:],
                    hT_r.rearrange("(mc p) s -> p mc s", p=P)[:, :, nsl])
                h2b = scr.tile([P, 2, 512], BF16, tag="h2b")
                for mc in range(2):
                    ps = pbx.tile([P, 512], F32, tag="big")
                    for kt in range(16):
                        nc.tensor.matmul(ps[:], wos[:, kt, mc], rhs[:, kt],
                                         start=(kt == 0), stop=(kt == 15))
                    nc.vector.tensor_add(h2[:, mc, nsl], ps[:], resid[:, mc])
                    nc.vector.tensor_copy(h2b[:, mc], h2[:, mc, nsl])
                nc.sync.dma_start(
                    ag3_in[hf].rearrange("(mc p) s -> p mc s", p=P)[
                        :, :, off:off + 512], h2b[:])

            def ag3_go(hf):
                nc.gpsimd.collective_compute(
                    "AllGather", mybir.AluOpType.bypass, replica_groups=RG,
                    ins=[ag3_in[hf]], outs=[ag3_out[hf]])

            with tc.tile_pool(name="pbq", bufs=2, space="PSUM") as pbq, \
                 tc.tile_pool(name="pbo", bufs=2, space="PSUM") as pbo:
                with nc.named_scope("stageB_kv"):
                    for pr in range(4):
                        psl = slice(pr * 512, (pr + 1) * 512)
                        rhs_c = sbr.tile([P, 4, 2, SS], BF16, tag="rhs1c", bufs=2)
                        for b in range(2):
                            blk = 2 * pr + b
                            nc.sync.dma_start(
                                rhs_c[:, :, b, :],
                                ag1a_out[blk * P:(blk + 1) * P, 0:4 * SS].rearrange(
                                    "p (kt s) -> p kt s", s=SS))
                            nc.sync.dma_start(
                                kpeT[:, blk * SS:(blk + 1) * SS],
                                ag1a_out[blk * P:blk * P + 64, 4 * SS:5 * SS])
                        # k_nope (dim-major)
                        for mc in range(2):
                            ps = pbq.tile([P, 512], F32, tag="sc")
                            for kt in range(4):
                                nc.tensor.matmul(
                                    ps[:], wkb[:, kt, mc],
                                    rhs_c[:, kt].rearrange("p b s -> p (b s)"),
                                    start=(kt == 0), stop=(kt == 3))
                            nc.vector.tensor_copy(kT[:, mc, psl], ps[:])
                        # V token-major: stationary = latent tile, moving = v-cols
                        for b in range(2):
                            for st2 in range(2):
                                stile = pr * 4 + b * 2 + st2
                                pv = pbo.tile([P, 2, P], F32, tag="o")
                                for kt in range(4):
                                    nc.tensor.matmul(
                                        pv[:].rearrange("p h v -> p (h v)"),
                                        rhs_c[:, kt, b, st2 * P:(st2 + 1) * P],
                                        wkb[:, kt, 2:4, :].rearrange("p h v -> p (h v)"),
                                        start=(kt == 0), stop=(kt == 3))
                                nc.vector.tensor_copy(v_tok[:, :, stile, :], pv[:])

                def qb_rope(pr):
                    """q_b + rope for token pair-block pr (512 tokens)."""
                    psl = slice(pr * 512, (pr + 1) * 512)
                    rhs_u = sbr.tile([P, 12, 2, SS], BF16, tag="rhs1u", bufs=2)
                    for b in range(2):
                        blk = 2 * pr + b
                        nc.sync.dma_start(
                            rhs_u[:, :, b, :],
                            ag1b_out[blk * P:(blk + 1) * P, :].rearrange(
                                "p (kt s) -> p kt s", s=SS))
                    qpe_raw = sbr.tile([P, 512], F32, tag="qpr")
                    for mc in range(3):
                        ps = pbq.tile([P, 512], F32, tag="sc")
                        for kt in range(12):
                            nc.tensor.matmul(
                                ps[:], wqb[:, kt, mc],
                                rhs_u[:, kt].rearrange("p b s -> p (b s)"),
                                start=(kt == 0), stop=(kt == 11))
                        if mc < 2:
                            nc.vector.tensor_copy(qT[:, mc, psl], ps[:])
                        else:
                            nc.vector.tensor_copy(qpe_raw[:], ps[:])
                    qsw = sbr.tile([P, 512], F32, tag="qsw")
                    for qq in range(2):
                        b0 = qq * 64
                        nc.sync.dma_start(qsw[b0:b0 + 32, :],
                                          qpe_raw[b0 + 32:b0 + 64, :])
                        nc.sync.dma_start(qsw[b0 + 32:b0 + 64, :],
                                          qpe_raw[b0:b0 + 32, :])
                    qpe_rot = sbr.tile([P, 512], BF16, tag="qro")
                    nc.vector.tensor_mul(qpe_rot[:], qpe_raw[:], cos_t[:, psl])
                    t1r = sbr.tile([P, 512], F32, tag="qt1")
                    nc.vector.tensor_mul(t1r[:], qsw[:], sin_t[:, psl])
                    nc.vector.tensor_add(qpe_rot[:], qpe_rot[:], t1r[:])
                    nc.sync.dma_start(qpe2[:, 0, psl], qpe_rot[0:64, :])
                    nc.sync.dma_start(qpe2[:, 1, psl], qpe_rot[64:128, :])

                def attn_chunk(qc):
                    qsl = slice(qc * 512, (qc + 1) * 512)
                    nkt = 4 * qc + 4
                    for h in range(2):
                        o_ps = pbo.tile([P, 512], F32, tag="o")
                        d_ps = pbx.tile([1, 512], F32, tag="acc")
                        for kt in range(nkt):
                            ksl = slice(kt * P, (kt + 1) * P)
                            sc_ps = pbq.tile([P, 512], F32, tag="sc")
                            nc.tensor.matmul(sc_ps[:], kT[:, h, ksl],
                                             qT[:, h, qsl], start=True, stop=False)
                            nc.tensor.matmul(sc_ps[:], kpeT[:, ksl],
                                             qpe2[:, h, qsl], start=False, stop=True)
                            j = kt - 4 * qc
                            if j >= 0:
                                nc.vector.tensor_add(sc_ps[:], sc_ps[:],
                                                     mask_t[:, j])
                            es = sbe.tile([P, 512], BF16, tag="es", bufs=4)
                            nc.scalar.activation(es[:], sc_ps[:], AF.Exp)
                            nc.tensor.matmul(o_ps[:], v_tok[:, h, kt], es[:],
                                             start=(kt == 0), stop=(kt == nkt - 1))
                            nc.tensor.matmul(d_ps[:], ones_r[:], es[:],
                                             start=(kt == 0), stop=(kt == nkt - 1))
                        rec = sbe.tile([1, 512], F32R, tag="rec", bufs=2)
                        with nc.allow_low_precision(
                                reason="f32r rounding of softmax denom"):
                            nc.vector.reciprocal(rec[:], d_ps[:])
                        rb_ps = pbx.tile([P, 512], F32, tag="big")
                        nc.tensor.matmul(rb_ps[:], ones_k1[:], rec[:],
                                         start=True, stop=True)
                        recb = sbe.tile([P, 512], F32, tag="recb", bufs=2)
                        nc.vector.tensor_copy(recb[:], rb_ps[:])
                        nc.vector.tensor_mul(oT[:, h, qsl], o_ps[:], recb[:])

                def ag2_go(bi, o_off, w):
                    nc.sync.dma_start(
                        ag2_in[bi].rearrange("(mc p) s -> p mc s", p=P),
                        oT[:, :, o_off:o_off + w])
                    nc.gpsimd.collective_compute(
                        "AllGather", mybir.AluOpType.bypass, replica_groups=RG,
                        ins=[ag2_in[bi]], outs=[ag2_out[bi]])

                with nc.named_scope("stageB_attn"):
                    qb_rope(3)
                    qb_rope(2)
                    attn_chunk(3)
                    qb_rope(1)
                    attn_chunk(2)
                    ag2_go(0, 1024, 1024)
                    qb_rope(0)
                    attn_chunk(1)
                    ag2_go(1, 512, 512)
                    oproj(3)
                    oproj(2)
                    attn_chunk(0)
                    ag2_go(2, 0, 512)
                    ag3_go(1)
                    oproj(1)
                    oproj(0)
                    ag3_go(0)

        # ================= Stage D: post-LN + MLP, chunk pipelined =============
        with tc.tile_pool(name="wmlp", bufs=1) as wmlp, \
             tc.tile_pool(name="wstr", bufs=1) as wstr, \
             tc.tile_pool(name="smy", bufs=1) as smy, \
             tc.tile_pool(name="sdd", bufs=2) as sdd, \
             tc.tile_pool(name="pgu", bufs=1, space="PSUM") as pgu:
            with nc.named_scope("stageD"):
                wds = wmlp.tile([P, 8, 16, P], BF16)
                nc.sync.dma_start(wds[:], wd_t[:])

                def hy_fetch(j):
                    hf = j // 2
                    off = j * 512 - hf * 1024
                    hy = smy.tile([P, 16, 512], BF16, tag="hy", bufs=3)
                    nc.sync.dma_start(
                        hy[:],
                        ag3_out[hf].rearrange("(kt p) s -> p kt s", p=P)[
                            :, :, off:off + 512])
                    return hy

                def stats(hy):
                    """Local post-LN stats; scales hy into y in place."""
                    m4 = pbx.tile([1, 512], F32, tag="acc")
                    for half in range(2):
                        sqh = smy.tile([P, 8, 512], BF16, tag="sqh", bufs=1)
                        nc.vector.tensor_mul(sqh[:], hy[:, half * 8:half * 8 + 8],
                                             hy[:, half * 8:half * 8 + 8])
                        for kt in range(8):
                            nc.tensor.matmul(m4[:], ones_r[:], sqh[:, kt],
                                             start=(half == 0 and kt == 0),
                                             stop=(half == 1 and kt == 7))
                    r4s = smy.tile([1, 512], F32, tag="r4s", bufs=2)
                    nc.scalar.activation(r4s[:], m4[:], AF.Sqrt,
                                         scale=1.0 / H, bias=eps_t[:1])
                    r4 = smy.tile([1, 512], F32R, tag="r4", bufs=2)
                    with nc.allow_low_precision(reason="f32r rounding of rms scale"):
                        nc.vector.reciprocal(r4[:], r4s[:])
                    r4bp = pbx.tile([P, 512], F32, tag="big")
                    nc.tensor.matmul(r4bp[:], ones_k1[:], r4[:],
                                     start=True, stop=True)
                    r4b = smy.tile([P, 512], F32, tag="r4b", bufs=2)
                    nc.vector.tensor_copy(r4b[:], r4bp[:])
                    nc.vector.tensor_mul(hy[:], hy[:],
                                         r4b[:, None, :].to_broadcast([P, 16, 512]))
                    return hy

                def gateup(y, y_off, cw):
                    """gate/up on y[:, :, y_off:y_off+cw], streaming weights."""
                    act = smy.tile([P, 8, 512], BF16, tag="act", bufs=2,
                                   name="act")[:, :, :cw]
                    for m in range(8):
                        wgm = wstr.tile([P, 16, P], BF16, tag="wg", bufs=4)
                        nc.sync.dma_start(wgm[:], wg_t[m])
                        wum = wstr.tile([P, 16, P], BF16, tag="wu", bufs=4)
                        nc.sync.dma_start(wum[:], wu_t[m])
                        gp = pgu.tile([P, 512], F32, tag=f"g{m % 2}",
                                      name="gp")[:, :cw]
                        up = pgu.tile([P, 512], F32, tag=f"u{m % 2}",
                                      name="up")[:, :cw]
                        for kt in range(16):
                            nc.tensor.matmul(gp[:], wgm[:, kt],
                                             y[:, kt, y_off:y_off + cw],
                                             start=(kt == 0), stop=(kt == 15))
                            nc.tensor.matmul(up[:], wum[:, kt],
                                             y[:, kt, y_off:y_off + cw],
                                             start=(kt == 0), stop=(kt == 15))
                        gsil = sdd.tile([P, 512], BF16, tag="gsil",
                                        name="gsil")[:, :cw]
                        nc.scalar.activation(gsil[:], gp[:], AF.Silu)
                        nc.vector.tensor_mul(act[:, m], gsil[:], up[:])
                    return act

                def down(act, act_off, ri):
                    """down-proj of act[:, :, act_off:act_off+w] -> RS chunk ri."""
                    c0, cw = RS_CH[ri]
                    nsl = slice(c0, c0 + cw)
                    for q in range(4):
                        dn = sdd.tile([P, 4, 512], BF16, tag="dn",
                                      name="dn")[:, :, :cw]
                        for s in range(4):
                            mc = 4 * q + s
                            ps = pbx.tile([P, 512], F32, tag="big",
                                          name="dps")[:, :cw]
                            for kt in range(8):
                                nc.tensor.matmul(
                                    ps[:], wds[:, kt, mc],
                                    act[:, kt, act_off:act_off + cw],
                                    start=(kt == 0), stop=(kt == 7))
                            if mc % 2 == 0:
                                nc.vector.tensor_copy(dn[:, s], ps[:])
                            else:
                                nc.scalar.activation(dn[:, s], ps[:], AF.Copy)
                        nc.sync.dma_start(
                            rs_in[ri][4 * q * P:4 * (q + 1) * P, :].rearrange(
                                "(q p) s -> p q s", p=P), dn[:])
                    nc.gpsimd.collective_compute(
                        "ReduceScatter", mybir.AluOpType.add, replica_groups=RG,
                        ins=[rs_in[ri]], outs=[rs_out[ri]])
                    fin = sdd.tile([P, 2, 512], BF16, tag="fin",
                                   name="fin")[:, :, :cw]
                    nc.sync.dma_start(
                        fin[:], rs_out[ri].rearrange("(mc p) s -> p mc s", p=P))
                    fino = sdd.tile([P, 2, 512], F32, tag="fino", name="fino",
                                    bufs=1)[:, :, :cw]
                    nc.vector.tensor_add(fino[:], fin[:], h2[:, :, nsl])
                    nc.sync.dma_start(
                        outT.rearrange("(mc p) s -> p mc s", p=P)[:, :, nsl],
                        fino[:])

                hy3 = hy_fetch(3)
                y3 = stats(hy3)
                hy2 = hy_fetch(2)
                act3 = gateup(y3, 0, 512)
                y2 = stats(hy2)
                down(act3, 0, 0)
                hy1 = hy_fetch(1)
                act2 = gateup(y2, 0, 512)
                y1 = stats(hy1)
                down(act2, 0, 1)
                hy0 = hy_fetch(0)
                act1 = gateup(y1, 0, 512)
                y0 = stats(hy0)
                down(act1, 0, 2)
                act0a = gateup(y0, 0, 256)
                down(act0a, 0, 3)
                act0b = gateup(y0, 256, 256)
                down(act0b, 0, 4)

    nc.compile()
    _CACHE["nc"] = nc
    return nc


def _host_prep(inputs):
    import ml_dtypes
    bf16 = ml_dtypes.bfloat16
    inp = {k: np.asarray(v) for k, v in inputs.items()}
    hidden = inp["hidden_states"].reshape(S, H).astype(np.float32)
    pos = inp["position_ids"].reshape(S).astype(np.int64)
    cosT = inp["cos"][pos].T.astype(np.float32)
    sinT = inp["sin"][pos].T.astype(np.float32)
    wq_a = (inp["wq_a"] * inp["in_ln"][:, None]).astype(np.float32)
    wkv_a = (inp["wkv_a"] * inp["in_ln"][:, None]).astype(np.float32)
    wq_b = (inp["wq_b"] * inp["q_a_ln"][:, None]).astype(np.float32)
    wkv_b = (inp["wkv_b"] * inp["kv_a_ln"][:, None]).astype(np.float32)
    wg = (inp["w_gate"] * inp["post_ln"][:, None]).astype(np.float32)
    wu = (inp["w_up"] * inp["post_ln"][:, None]).astype(np.float32)
    wd = inp["w_down"].astype(np.float32)
    wo = inp["wo"].astype(np.float32)

    de = np.empty(ROPE, np.int64)
    de[:32] = np.arange(32) * 2
    de[32:] = np.arange(32) * 2 + 1
    wkv_a = np.concatenate([wkv_a[:, :KVLR], wkv_a[:, KVLR:][:, de]], axis=1)
    wq_b = wq_b.reshape(QLR, NH, QHD)
    wkv_b = wkv_b.reshape(KVLR, NH, NOPE + VHD)

    hT = hidden.T.copy()
    sin_sg = np.concatenate([-sinT[:32], sinT[32:]], axis=0)    # signed for swap trick
    cossin = np.concatenate([cosT, cosT, sin_sg, sin_sg], axis=0)  # (256, S)
    ki = np.arange(P)[:, None]
    qi = np.arange(512)[None, :]
    dmask = np.stack([np.where(qi >= j * P + ki, 0.0, -1e30).astype(np.float32)
                      for j in range(4)], axis=1)               # (128, 4, 512)

    wq_a_t = _tile_w(wq_a)
    wkv_a_t = _tile_w(wkv_a)

    in_maps = []
    for c in range(NC):
        h0, h1 = 2 * c, 2 * c + 1
        qb = np.concatenate([
            wq_b[:, h0, :NOPE], wq_b[:, h1, :NOPE],
            wq_b[:, h0, NOPE:][:, de], wq_b[:, h1, NOPE:][:, de]], axis=1) * SCALE
        kb = np.concatenate([
            wkv_b[:, h0, :NOPE], wkv_b[:, h1, :NOPE],
            wkv_b[:, h0, NOPE:], wkv_b[:, h1, NOPE:]], axis=1)
        ssl = slice(c * SS, (c + 1) * SS)
        cs_sh = np.concatenate([cosT[:, ssl], sin_sg[:, ssl]], axis=0)
        in_maps.append({
            "hT_s": np.ascontiguousarray(hT[:, ssl]),
            "hT_r": np.ascontiguousarray(hT[ssl, :]),
            "wq_a_t": np.ascontiguousarray(
                wq_a_t.transpose(1, 2, 0, 3)).astype(bf16),
            "wkv_a_t": np.ascontiguousarray(
                wkv_a_t.transpose(1, 2, 0, 3)).astype(bf16),
            "wq_b_t": np.ascontiguousarray(_tile_w(
                qb.astype(np.float32)).transpose(2, 0, 1, 3)).astype(bf16),
            "wkv_b_t": np.ascontiguousarray(_tile_w(
                kb.astype(np.float32)).transpose(2, 0, 1, 3)).astype(bf16),
            "wo_t": np.ascontiguousarray(_tile_w(np.ascontiguousarray(
                wo[:, ssl])).transpose(2, 0, 1, 3)).astype(bf16),
            "wg_t": np.ascontiguousarray(_tile_w(
                wg[:, c * FFS:(c + 1) * FFS]).transpose(1, 2, 0, 3)).astype(bf16),
            "wu_t": np.ascontiguousarray(_tile_w(
                wu[:, c * FFS:(c + 1) * FFS]).transpose(1, 2, 0, 3)).astype(bf16),
            "wd_t": np.ascontiguousarray(_tile_w(
                wd[c * FFS:(c + 1) * FFS, :]).transpose(2, 0, 1, 3)).astype(bf16),
            "cossin": cossin,
            "cs_sh": np.ascontiguousarray(cs_sh),
            "dmask": dmask,
        })
    return in_maps


_LAST_RESULT = {}


def kernel(**inputs) -> np.ndarray:
    from concourse.bass_utils import run_bass_kernel_spmd
    nc = _build()
    in_maps = _host_prep(inputs)
    kwargs = {}
    if TRACE:
        import sys, types
        if "antenv.axon_hooks" not in sys.modules:
            try:
                from trn_agent_boot.trn_boot import _ntff_profile_via_ctypes
                mod = types.ModuleType("antenv.axon_hooks")
                _hook = _ntff_profile_via_ctypes('/opt/axon/libaxon_pjrt.so')
                mod.get_axon_ntff_profile_hook = lambda: _hook
                mod.set_axon_ntff_profile_hook = lambda h: None
                sys.modules["antenv.axon_hooks"] = mod
                import antenv
                antenv.axon_hooks = mod
            except Exception:
                pass
        kwargs["trace"] = True
    res = run_bass_kernel_spmd(nc, in_maps, list(range(NC)), **kwargs)
    _LAST_RESULT["res"] = res
    outT = np.concatenate([res.results[c]["outT"] for c in range(NC)], axis=0)
    return np.ascontiguousarray(outT.T)[None].astype(np.float32)


# revision 12
# speedup vs baseline: 1.0920x; 1.0046x over previous
"""DeepseekV3 decoder layer on 8 Trainium2 NeuronCores (Bass/Tile).

Sharding: sequence-parallel low-rank projections (one AllGather per latent
group), tensor-parallel heads for q_b/kv_b/attention (2 heads/core,
transposed-score layout), AllGather of head outputs, output-feature-sharded
o_proj + residual, AllGather of the raw post-attention hidden (post-LN stats
are recomputed locally on every core - no stats AllReduce), FF-sharded MLP
with per-chunk ReduceScatter.

Schedule: attention query chunks run in descending size order (3,2,1,0) so
the smallest chunk is last; q_b/rope for chunk j-1 and o_proj/AG3 for chunk
j+1 are interleaved between attention chunks; the MLP consumes AG3 chunks in
arrival order and the final down-proj/ReduceScatter is split 2x256 to shrink
the tail.

RMS scale-invariance: RMS() of a row of (x @ W) is independent of a
per-token scale on x, so the q/kv latent projections run directly on raw
bf16 x with no input RMS on the critical path; only k_pe (64 rows) needs
the 1/rms(x) factor. RMS/ln weights and the rope de-interleave are folded
into the weights host-side.
"""

import numpy as np

B, S, H = 1, 2048, 2048
NH, NOPE, ROPE, VHD = 16, 128, 64, 128
QHD = NOPE + ROPE
QLR, KVLR, FF = 1536, 512, 8192
SCALE = QHD ** -0.5
EPS = 1e-6
NC = 8
SS = S // NC            # 256: sequence / output-feature shard
FFS = FF // NC          # 1024: FF shard
P = 128

TRACE = False           # test.py sets kernel.TRACE = True for profiling

_CACHE = {}


def _tile_w(w):
    """[K, M] -> [K/128, ceil(M/128), 128, 128] contiguous blocks (zero-pad M)."""
    K, M = w.shape
    mc = -(-M // P)
    out = np.zeros((K // P, mc, P, P), np.float32)
    wp = np.zeros((K, mc * P), np.float32)
    wp[:, :M] = w
    for kt in range(K // P):
        for m in range(mc):
            out[kt, m] = wp[kt * P:(kt + 1) * P, m * P:(m + 1) * P]
    return out


def _build():
    if "nc" in _CACHE:
        return _CACHE["nc"]
    import concourse.mybir as mybir
    import concourse.tile as tile
    from concourse import bacc

    F32 = mybir.dt.float32
    F32R = mybir.dt.float32r
    BF16 = mybir.dt.bfloat16
    AF = mybir.ActivationFunctionType

    nc = bacc.Bacc("TRN2", target_bir_lowering=False, debug=False, num_devices=NC)

    def inp(name, shape, dt=F32):
        return nc.dram_tensor(name, list(shape), dt, kind="ExternalInput").ap()

    hT_s = inp("hT_s", [H, SS])
    hT_r = inp("hT_r", [SS, S])
    wq_a_t = inp("wq_a_t", [12, P, 16, P], BF16)
    wkv_a_t = inp("wkv_a_t", [5, P, 16, P], BF16)
    wq_b_t = inp("wq_b_t", [P, 12, 3, P], BF16)
    wkv_b_t = inp("wkv_b_t", [P, 4, 4, P], BF16)
    wo_t = inp("wo_t", [P, 16, 2, P], BF16)
    wg_t = inp("wg_t", [8, P, 16, P], BF16)
    wu_t = inp("wu_t", [8, P, 16, P], BF16)
    wd_t = inp("wd_t", [P, 8, 16, P], BF16)
    cossin = inp("cossin", [2 * P, S])        # rows 0:128 [cosT;cosT], 128:256 [sinT;sinT]
    cs_sh = inp("cs_sh", [P, SS])             # rows 0:64 cosT, 64:128 signed sinT (own shard)
    dmask = inp("dmask", [P, 4, 512])
    outT = nc.dram_tensor("outT", [SS, S], F32, kind="ExternalOutput").ap()

    RG = [list(range(NC))]

    from contextlib import ExitStack
    with tile.TileContext(nc) as tc, ExitStack() as _stack:
        cpool = _stack.enter_context(tc.tile_pool(name="const", bufs=1))
        dpool = _stack.enter_context(tc.tile_pool(name="dram", bufs=1, space="DRAM"))
        perm = _stack.enter_context(tc.tile_pool(name="perm", bufs=1))

        ag1a_in = dpool.tile([P, 5 * SS], BF16)
        ag1a_out = dpool.tile([NC * P, 5 * SS], BF16, addr_space="Shared")
        ag1b_in = dpool.tile([P, 12 * SS], BF16)
        ag1b_out = dpool.tile([NC * P, 12 * SS], BF16, addr_space="Shared")
        _ag2w = [1024, 512, 512]          # h1 (chunks 3+2), c1, c0
        ag2_in = [dpool.tile([2 * VHD, w], BF16, name=f"ag2_in{j}")
                  for j, w in enumerate(_ag2w)]
        ag2_out = [dpool.tile([NH * VHD, w], BF16, addr_space="Shared",
                              name=f"ag2_out{j}") for j, w in enumerate(_ag2w)]
        # per attn chunk j: (ag2 buffer index, col offset, oT col offset)
        AG2_MAP = {3: (0, 512, 1024), 2: (0, 0, 1024), 1: (1, 0, 512),
                   0: (2, 0, 0)}
        ag3_in = [dpool.tile([SS, 1024], BF16, name=f"ag3_in{j}") for j in range(2)]
        ag3_out = [dpool.tile([H, 1024], BF16, addr_space="Shared",
                              name=f"ag3_out{j}") for j in range(2)]
        # rs chunks: (outT column offset, width), in processing order
        RS_CH = [(1536, 512), (1024, 512), (512, 512), (0, 256), (256, 256)]
        rs_in = [dpool.tile([H, w], BF16, name=f"rs_in{j}")
                 for j, (c0, w) in enumerate(RS_CH)]
        rs_out = [dpool.tile([SS, w], BF16, name=f"rs_out{j}")
                  for j, (c0, w) in enumerate(RS_CH)]

        ones_f = cpool.tile([P, 1], F32)
        nc.vector.memset(ones_f[:], 1.0)
        ones_r = cpool.tile([P, 1], BF16)
        nc.vector.tensor_copy(ones_r[:], ones_f[:])
        eps_t = cpool.tile([P, 1], F32)
        nc.vector.memset(eps_t[:], EPS)
        ones_k1f = cpool.tile([1, P], F32)
        nc.vector.memset(ones_k1f[:], 1.0)
        ones_k1 = cpool.tile([1, P], F32R)
        nc.vector.tensor_copy(ones_k1[:], ones_k1f[:])

        h2 = perm.tile([P, 2, S], F32)        # post-attn hidden, own feature shard

        # shared psum pool for o_proj / stats / down accumulators (phases B+D)
        pbx = _stack.enter_context(tc.tile_pool(name="pbx", bufs=2, space="PSUM"))

        # ================= Stage A: seq-shard low-rank path =================
        with tc.tile_pool(name="sa", bufs=1) as sa, \
             tc.tile_pool(name="saw", bufs=5) as saw, \
             tc.tile_pool(name="pap", bufs=2, space="PSUM") as pa:
            with nc.named_scope("stageA"):
                xs = sa.tile([P, 16, SS], F32)
                nc.sync.dma_start(xs[:], hT_s.rearrange("(kt p) s -> p kt s", p=P))
                xb = sa.tile([P, 16, SS], BF16)
                nc.vector.tensor_copy(xb[:], xs[:])

                # kv latents on raw x (RMS scale-invariance)
                cvs = sa.tile([P, 5, SS], F32)
                for mc in range(5):
                    wt = saw.tile([P, 16, P], BF16, tag="aw")
                    nc.sync.dma_start(wt[:], wkv_a_t[mc])
                    ps = pa.tile([P, SS], F32, tag="amm")
                    for kt in range(16):
                        nc.tensor.matmul(ps[:], wt[:, kt], xb[:, kt],
                                         start=(kt == 0), stop=(kt == 15))
                    nc.vector.tensor_copy(cvs[:, mc], ps[:])

                # kv_a RMS (on raw latents; the 1/rms(x) factor cancels)
                sq3 = sa.tile([P, 4, SS], BF16)
                nc.vector.tensor_mul(sq3[:], cvs[:, :4], cvs[:, :4])
                msq3 = pa.tile([1, SS], F32, tag="acc", bufs=1)
                for mc in range(4):
                    nc.tensor.matmul(msq3[:], ones_r[:], sq3[:, mc],
                                     start=(mc == 0), stop=(mc == 3))
                r3s = sa.tile([1, SS], F32)
                nc.scalar.activation(r3s[:], msq3[:], AF.Sqrt, scale=1.0 / KVLR, bias=eps_t[:1])
                r3 = sa.tile([1, SS], F32R)
                with nc.allow_low_precision(reason="f32r rounding of rms scale"):
                    nc.vector.reciprocal(r3[:], r3s[:])
                r3bp = pa.tile([P, SS], F32, tag="rb", bufs=1)
                nc.tensor.matmul(r3bp[:], ones_k1[:], r3[:], start=True, stop=True)
                r3b = sa.tile([P, SS], F32)
                nc.vector.tensor_copy(r3b[:], r3bp[:])
                ckn = sa.tile([P, 4, SS], BF16)
                nc.vector.tensor_mul(ckn[:], cvs[:, :4],
                                     r3b[:, None, :].to_broadcast([P, 4, SS]))

                # rms(x) for the k_pe rows only
                sqx = sa.tile([P, 16, SS], BF16)
                nc.vector.tensor_mul(sqx[:], xb[:], xb[:])
                msq1 = pa.tile([1, SS], F32, tag="acc", bufs=1)
                for kt in range(16):
                    nc.tensor.matmul(msq1[:], ones_r[:], sqx[:, kt],
                                     start=(kt == 0), stop=(kt == 15))
                r1s = sa.tile([1, SS], F32)
                nc.scalar.activation(r1s[:], msq1[:], AF.Sqrt, scale=1.0 / H, bias=eps_t[:1])
                r1 = sa.tile([1, SS], F32R)
                with nc.allow_low_precision(reason="f32r rounding of rms scale"):
                    nc.vector.reciprocal(r1[:], r1s[:])
                r1bp = pa.tile([64, SS], F32, tag="rb", bufs=1)
                nc.tensor.matmul(r1bp[:], ones_k1[:, :64], r1[:], start=True, stop=True)
                r1b = sa.tile([64, SS], F32)
                nc.vector.tensor_copy(r1b[:], r1bp[:])

                # k_pe rope on cvs[:64, 4] (cs_sh rows 0:64 cos, 64:128 signed sin)
                cos_sh = sa.tile([64, SS], F32)
                nc.sync.dma_start(cos_sh[:], cs_sh[0:64, :])
                sin_sh = sa.tile([64, SS], F32)
                nc.sync.dma_start(sin_sh[:], cs_sh[64:128, :])
                ksw = sa.tile([64, SS], F32)
                nc.sync.dma_start(ksw[0:32, :], cvs[32:64, 4])
                nc.sync.dma_start(ksw[32:64, :], cvs[0:32, 4])
                kro = sa.tile([64, SS], F32)
                nc.vector.tensor_mul(kro[:], cvs[:64, 4], cos_sh[:])
                t1 = sa.tile([64, SS], F32)
                nc.vector.tensor_mul(t1[:], ksw[:], sin_sh[:])
                nc.vector.tensor_add(kro[:], kro[:], t1[:])
                kpe_n = sa.tile([64, SS], BF16)
                nc.vector.tensor_mul(kpe_n[:], kro[:], r1b[:])

                nc.sync.dma_start(
                    ag1a_in[:, 0:4 * SS].rearrange("p (kt s) -> p kt s", s=SS),
                    ckn[:])
                nc.sync.dma_start(ag1a_in[:64, 4 * SS:5 * SS], kpe_n[:])
                nc.gpsimd.collective_compute(
                    "AllGather", mybir.AluOpType.bypass, replica_groups=RG,
                    ins=[ag1a_in], outs=[ag1a_out])

                # q latents on raw x
                us = sa.tile([P, 12, SS], F32)
                for mc in range(12):
                    wt = saw.tile([P, 16, P], BF16, tag="aw")
                    nc.sync.dma_start(wt[:], wq_a_t[mc])
                    ps = pa.tile([P, SS], F32, tag="amm")
                    for kt in range(16):
                        nc.tensor.matmul(ps[:], wt[:, kt], xb[:, kt],
                                         start=(kt == 0), stop=(kt == 15))
                    nc.vector.tensor_copy(us[:, mc], ps[:])

                sq2 = sa.tile([P, 12, SS], BF16)
                nc.vector.tensor_mul(sq2[:], us[:], us[:])
                msq2 = pa.tile([1, SS], F32, tag="acc", bufs=1)
                for mc in range(12):
                    nc.tensor.matmul(msq2[:], ones_r[:], sq2[:, mc],
                                     start=(mc == 0), stop=(mc == 11))
                r2s = sa.tile([1, SS], F32)
                nc.scalar.activation(r2s[:], msq2[:], AF.Sqrt, scale=1.0 / QLR, bias=eps_t[:1])
                r2 = sa.tile([1, SS], F32R)
                with nc.allow_low_precision(reason="f32r rounding of rms scale"):
                    nc.vector.reciprocal(r2[:], r2s[:])
                r2bp = pa.tile([P, SS], F32, tag="rb", bufs=1)
                nc.tensor.matmul(r2bp[:], ones_k1[:], r2[:], start=True, stop=True)
                r2b = sa.tile([P, SS], F32)
                nc.vector.tensor_copy(r2b[:], r2bp[:])
                un = sa.tile([P, 12, SS], BF16)
                nc.vector.tensor_mul(un[:], us[:],
                                     r2b[:, None, :].to_broadcast([P, 12, SS]))
                nc.sync.dma_start(
                    ag1b_in.rearrange("p (kt s) -> p kt s", s=SS), un[:])
                nc.gpsimd.collective_compute(
                    "AllGather", mybir.AluOpType.bypass, replica_groups=RG,
                    ins=[ag1b_in], outs=[ag1b_out])

        # ===== Stage B: kv_b all blocks, per-chunk q_b/rope + attention =====
        with tc.tile_pool(name="sb2", bufs=1) as sb2, \
             tc.tile_pool(name="sbr", bufs=1) as sbr, \
             tc.tile_pool(name="sbe", bufs=1) as sbe, \
             tc.tile_pool(name="scr", bufs=2) as scr:
            kT = sb2.tile([P, 2, S], BF16)
            kpeT = sb2.tile([64, S], BF16)
            v_tok = sb2.tile([P, 2, 16, P], BF16)
            qT = sb2.tile([P, 2, S], BF16)
            qpe2 = sb2.tile([64, 2, S], BF16)
            oT = sb2.tile([P, 2, S], BF16)
            wkb = sb2.tile([P, 4, 4, P], BF16)
            wqb = sb2.tile([P, 12, 3, P], BF16)
            wos = sb2.tile([P, 16, 2, P], BF16)
            cos_t = sb2.tile([P, S], F32)
            sin_t = sb2.tile([P, S], F32)
            mask_t = sb2.tile([P, 4, 512], F32)
            nc.sync.dma_start(wkb[:], wkv_b_t[:])
            nc.sync.dma_start(wqb[:], wq_b_t[:])
            nc.sync.dma_start(cos_t[:], cossin[0:P, :])
            nc.sync.dma_start(sin_t[:], cossin[P:2 * P, :])
            nc.sync.dma_start(mask_t[:], dmask[:, :, :])
            nc.sync.dma_start(wos[:], wo_t[:])

            def oproj(j):
                """o_proj chunk j + residual -> h2, bf16 h2 into ag3_in half."""
                hf = j // 2
                off = j * 512 - hf * 1024
                bi, boff, _ = AG2_MAP[j]
                nsl = slice(j * 512, (j + 1) * 512)
                rhs = scr.tile([P, 16, 512], BF16, tag="rhs2")
                nc.sync.dma_start(
                    rhs[:],
                    ag2_out[bi].rearrange("(kt p) s -> p kt s", p=P)[
                        :, :, boff:boff + 512])
                resid = scr.tile([P, 2, 512], F32, tag="resid", bufs=1)
                nc.sync.dma_start(
                    resid[:],
                    hT_r.rearrange("(mc p) s -> p mc s", p=P)[:, :, nsl])
                h2b = scr.tile([P, 2, 512], BF16, tag="h2b")
                for mc in range(2):
                    ps = pbx.tile([P, 512], F32, tag="big")
                    for kt in range(16):
                        nc.tensor.matmul(ps[:], wos[:, kt, mc], rhs[:, kt],
                                         start=(kt == 0), stop=(kt == 15))
                    nc.vector.tensor_add(h2[:, mc, nsl], ps[:], resid[:, mc])
                    nc.vector.tensor_copy(h2b[:, mc], h2[:, mc, nsl])
                nc.sync.dma_start(
                    ag3_in[hf].rearrange("(mc p) s -> p mc s", p=P)[
                        :, :, off:off + 512], h2b[:])

            def ag3_go(hf):
                nc.gpsimd.collective_compute(
                    "AllGather", mybir.AluOpType.bypass, replica_groups=RG,
                    ins=[ag3_in[hf]], outs=[ag3_out[hf]])

            with tc.tile_pool(name="pbq", bufs=2, space="PSUM") as pbq, \
                 tc.tile_pool(name="pbo", bufs=2, space="PSUM") as pbo:
                with nc.named_scope("stageB_kv"):
                    for pr in range(4):
                        psl = slice(pr * 512, (pr + 1) * 512)
                        rhs_c = sbr.tile([P, 4, 2, SS], BF16, tag="rhs1c", bufs=2)
                        for b in range(2):
                            blk = 2 * pr + b
                            nc.sync.dma_start(
                                rhs_c[:, :, b, :],
                                ag1a_out[blk * P:(blk + 1) * P, 0:4 * SS].rearrange(
                                    "p (kt s) -> p kt s", s=SS))
                            nc.sync.dma_start(
                                kpeT[:, blk * SS:(blk + 1) * SS],
                                ag1a_out[blk * P:blk * P + 64, 4 * SS:5 * SS])
                        # k_nope (dim-major)
                        for mc in range(2):
                            ps = pbq.tile([P, 512], F32, tag="sc")
                            for kt in range(4):
                                nc.tensor.matmul(
                                    ps[:], wkb[:, kt, mc],
                                    rhs_c[:, kt].rearrange("p b s -> p (b s)"),
                                    start=(kt == 0), stop=(kt == 3))
                            nc.vector.tensor_copy(kT[:, mc, psl], ps[:])
                        # V token-major: stationary = latent tile, moving = v-cols
                        for b in range(2):
                            for st2 in range(2):
                                stile = pr * 4 + b * 2 + st2
                                pv = pbo.tile([P, 2, P], F32, tag="o")
                                for kt in range(4):
                                    nc.tensor.matmul(
                                        pv[:].rearrange("p h v -> p (h v)"),
                                        rhs_c[:, kt, b, st2 * P:(st2 + 1) * P],
                                        wkb[:, kt, 2:4, :].rearrange("p h v -> p (h v)"),
                                        start=(kt == 0), stop=(kt == 3))
                                nc.vector.tensor_copy(v_tok[:, :, stile, :], pv[:])

                def qb_rope(pr):
                    """q_b + rope for token pair-block pr (512 tokens)."""
                    psl = slice(pr * 512, (pr + 1) * 512)
                    rhs_u = sbr.tile([P, 12, 2, SS], BF16, tag="rhs1u", bufs=2)
                    for b in range(2):
                        blk = 2 * pr + b
                        nc.sync.dma_start(
                            rhs_u[:, :, b, :],
                            ag1b_out[blk * P:(blk + 1) * P, :].rearrange(
                                "p (kt s) -> p kt s", s=SS))
                    qpe_raw = sbr.tile([P, 512], F32, tag="qpr")
                    for mc in range(3):
                        ps = pbq.tile([P, 512], F32, tag="sc")
                        for kt in range(12):
                            nc.tensor.matmul(
                                ps[:], wqb[:, kt, mc],
                                rhs_u[:, kt].rearrange("p b s -> p (b s)"),
                                start=(kt == 0), stop=(kt == 11))
                        if mc < 2:
                            nc.vector.tensor_copy(qT[:, mc, psl], ps[:])
                        else:
                            nc.vector.tensor_copy(qpe_raw[:], ps[:])
                    qsw = sbr.tile([P, 512], F32, tag="qsw")
                    for qq in range(2):
                        b0 = qq * 64
                        nc.sync.dma_start(qsw[b0:b0 + 32, :],
                                          qpe_raw[b0 + 32:b0 + 64, :])
                        nc.sync.dma_start(qsw[b0 + 32:b0 + 64, :],
                                          qpe_raw[b0:b0 + 32, :])
                    qpe_rot = sbr.tile([P, 512], BF16, tag="qro")
                    nc.vector.tensor_mul(qpe_rot[:], qpe_raw[:], cos_t[:, psl])
                    t1r = sbr.tile([P, 512], F32, tag="qt1")
                    nc.vector.tensor_mul(t1r[:], qsw[:], sin_t[:, psl])
                    nc.vector.tensor_add(qpe_rot[:], qpe_rot[:], t1r[:])
                    nc.sync.dma_start(qpe2[:, 0, psl], qpe_rot[0:64, :])
                    nc.sync.dma_start(qpe2[:, 1, psl], qpe_rot[64:128, :])

                def attn_chunk(qc):
                    qsl = slice(qc * 512, (qc + 1) * 512)
                    nkt = 4 * qc + 4
                    for h in range(2):
                        o_ps = pbo.tile([P, 512], F32, tag="o")
                        d_ps = pbx.tile([1, 512], F32, tag="acc")
                        for kt in range(nkt):
                            ksl = slice(kt * P, (kt + 1) * P)
                            sc_ps = pbq.tile([P, 512], F32, tag="sc")
                            nc.tensor.matmul(sc_ps[:], kT[:, h, ksl],
                                             qT[:, h, qsl], start=True, stop=False)
                            nc.tensor.matmul(sc_ps[:], kpeT[:, ksl],
                                             qpe2[:, h, qsl], start=False, stop=True)
                            j = kt - 4 * qc
                            if j >= 0:
                                nc.vector.tensor_add(sc_ps[:], sc_ps[:],
                                                     mask_t[:, j])
                            es = sbe.tile([P, 512], BF16, tag="es", bufs=4)
                            nc.scalar.activation(es[:], sc_ps[:], AF.Exp)
                            nc.tensor.matmul(o_ps[:], v_tok[:, h, kt], es[:],
                                             start=(kt == 0), stop=(kt == nkt - 1))
                            nc.tensor.matmul(d_ps[:], ones_r[:], es[:],
                                             start=(kt == 0), stop=(kt == nkt - 1))
                        rec = sbe.tile([1, 512], F32R, tag="rec", bufs=2)
                        with nc.allow_low_precision(
                                reason="f32r rounding of softmax denom"):
                            nc.vector.reciprocal(rec[:], d_ps[:])
                        rb_ps = pbx.tile([P, 512], F32, tag="big")
                        nc.tensor.matmul(rb_ps[:], ones_k1[:], rec[:],
                                         start=True, stop=True)
                        recb = sbe.tile([P, 512], F32, tag="recb", bufs=2)
                        nc.vector.tensor_copy(recb[:], rb_ps[:])
                        nc.vector.tensor_mul(oT[:, h, qsl], o_ps[:], recb[:])

                def ag2_go(bi, o_off, w):
                    nc.sync.dma_start(
                        ag2_in[bi].rearrange("(mc p) s -> p mc s", p=P),
                        oT[:, :, o_off:o_off + w])
                    nc.gpsimd.collective_compute(
                        "AllGather", mybir.AluOpType.bypass, replica_groups=RG,
                        ins=[ag2_in[bi]], outs=[ag2_out[bi]])

                with nc.named_scope("stageB_attn"):
                    qb_rope(3)
                    qb_rope(2)
                    attn_chunk(3)
                    qb_rope(1)
                    attn_chunk(2)
                    ag2_go(0, 1024, 1024)
                    qb_rope(0)
                    attn_chunk(1)
                    ag2_go(1, 512, 512)
                    oproj(3)
                    oproj(2)
                    attn_chunk(0)
                    ag2_go(2, 0, 512)
                    ag3_go(1)
                    oproj(1)
                    oproj(0)
                    ag3_go(0)

        # ================= Stage D: post-LN + MLP, chunk pipelined =============
        with tc.tile_pool(name="wmlp", bufs=1) as wmlp, \
             tc.tile_pool(name="wstr", bufs=1) as wstr, \
             tc.tile_pool(name="smy", bufs=1) as smy, \
             tc.tile_pool(name="sdd", bufs=2) as sdd, \
             tc.tile_pool(name="pgu", bufs=1, space="PSUM") as pgu:
            with nc.named_scope("stageD"):
                wds = wmlp.tile([P, 8, 16, P], BF16)
                nc.sync.dma_start(wds[:], wd_t[:])

                def hy_fetch(j):
                    hf = j // 2
                    off = j * 512 - hf * 1024
                    hy = smy.tile([P, 16, 512], BF16, tag="hy", bufs=3)
                    nc.sync.dma_start(
                        hy[:],
                        ag3_out[hf].rearrange("(kt p) s -> p kt s", p=P)[
                            :, :, off:off + 512])
                    return hy

                def stats(hy):
                    """Local post-LN stats; scales hy into y in place."""
                    m4 = pbx.tile([1, 512], F32, tag="acc")
                    for half in range(2):
                        sqh = smy.tile([P, 8, 512], BF16, tag="sqh", bufs=1)
                        nc.vector.tensor_mul(sqh[:], hy[:, half * 8:half * 8 + 8],
                                             hy[:, half * 8:half * 8 + 8])
                        for kt in range(8):
                            nc.tensor.matmul(m4[:], ones_r[:], sqh[:, kt],
                                             start=(half == 0 and kt == 0),
                                             stop=(half == 1 and kt == 7))
                    r4s = smy.tile([1, 512], F32, tag="r4s", bufs=2)
                    nc.scalar.activation(r4s[:], m4[:], AF.Sqrt,
                                         scale=1.0 / H, bias=eps_t[:1])
                    r4 = smy.tile([1, 512], F32R, tag="r4", bufs=2)
                    with nc.allow_low_precision(reason="f32r rounding of rms scale"):
                        nc.vector.reciprocal(r4[:], r4s[:])
                    r4bp = pbx.tile([P, 512], F32, tag="big")
                    nc.tensor.matmul(r4bp[:], ones_k1[:], r4[:],
                                     start=True, stop=True)
                    r4b = smy.tile([P, 512], F32, tag="r4b", bufs=2)
                    nc.vector.tensor_copy(r4b[:], r4bp[:])
                    nc.vector.tensor_mul(hy[:], hy[:],
                                         r4b[:, None, :].to_broadcast([P, 16, 512]))
                    return hy

                def gateup(y, y_off, cw):
                    """gate/up on y[:, :, y_off:y_off+cw], streaming weights."""
                    act = smy.tile([P, 8, 512], BF16, tag="act", bufs=2,
                                   name="act")[:, :, :cw]
                    for m in range(8):
                        wgm = wstr.tile([P, 16, P], BF16, tag="wg", bufs=4)
                        nc.sync.dma_start(wgm[:], wg_t[m])
                        wum = wstr.tile([P, 16, P], BF16, tag="wu", bufs=4)
                        nc.sync.dma_start(wum[:], wu_t[m])
                        gp = pgu.tile([P, 512], F32, tag=f"g{m % 2}",
                                      name="gp")[:, :cw]
                        up = pgu.tile([P, 512], F32, tag=f"u{m % 2}",
                                      name="up")[:, :cw]
                        for kt in range(16):
                            nc.tensor.matmul(gp[:], wgm[:, kt],
                                             y[:, kt, y_off:y_off + cw],
                                             start=(kt == 0), stop=(kt == 15))
                            nc.tensor.matmul(up[:], wum[:, kt],
                                             y[:, kt, y_off:y_off + cw],
                                             start=(kt == 0), stop=(kt == 15))
                        gsil = sdd.tile([P, 512], BF16, tag="gsil",
                                        name="gsil")[:, :cw]
                        nc.scalar.activation(gsil[:], gp[:], AF.Silu)
                        nc.vector.tensor_mul(act[:, m], gsil[:], up[:])
                    return act

                def down(act, act_off, ri):
                    """down-proj of act[:, :, act_off:act_off+w] -> RS chunk ri."""
                    c0, cw = RS_CH[ri]
                    nsl = slice(c0, c0 + cw)
                    for q in range(4):
                        dn = sdd.tile([P, 4, 512], BF16, tag="dn",
                                      name="dn")[:, :, :cw]
                        for s in range(4):
                            mc = 4 * q + s
                            ps = pbx.tile([P, 512], F32, tag="big",
                                          name="dps")[:, :cw]
                            for kt in range(8):
                                nc.tensor.matmul(
                                    ps[:], wds[:, kt, mc],
                                    act[:, kt, act_off:act_off + cw],
                                    start=(kt == 0), stop=(kt == 7))
                            if mc % 2 == 0:
                                nc.vector.tensor_copy(dn[:, s], ps[:])
                            else:
                                nc.scalar.activation(dn[:, s], ps[:], AF.Copy)
                        nc.sync.dma_start(
                            rs_in[ri][4 * q * P:4 * (q + 1) * P, :].rearrange(
                                "(q p) s -> p q s", p=P), dn[:])
                    nc.gpsimd.collective_compute(
                        "ReduceScatter", mybir.AluOpType.add, replica_groups=RG,
                        ins=[rs_in[ri]], outs=[rs_out[ri]])
                    fin = sdd.tile([P, 2, 512], BF16, tag="fin",
                                   name="fin")[:, :, :cw]
                    nc.sync.dma_start(
                        fin[:], rs_out[ri].rearrange("(mc p) s -> p mc s", p=P))
                    fino = sdd.tile([P, 2, 512], F32, tag="fino", name="fino",
                                    bufs=1)[:, :, :cw]
                    nc.vector.tensor_add(fino[:], fin[:], h2[:, :, nsl])
                    nc.sync.dma_start(
                        outT.rearrange("(mc p) s -> p mc s", p=P)[:, :, nsl],
                        fino[:])

                hy3 = hy_fetch(3)
                y3 = stats(hy3)
                hy2 = hy_fetch(2)
                act3 = gateup(y3, 0, 512)
                y2 = stats(hy2)
                down(act3, 0, 0)
                hy1 = hy_fetch(1)
                act2 = gateup(y2, 0, 512)
                y1 = stats(hy1)
                down(act2, 0, 1)
                hy0 = hy_fetch(0)
                act1 = gateup(y1, 0, 512)
                y0 = stats(hy0)
                down(act1, 0, 2)
                act0a = gateup(y0, 0, 256)
                down(act0a, 0, 3)
                act0b = gateup(y0, 256, 256)
                down(act0b, 0, 4)

    nc.compile()
    _CACHE["nc"] = nc
    return nc


def _host_prep(inputs):
    import ml_dtypes
    bf16 = ml_dtypes.bfloat16
    inp = {k: np.asarray(v) for k, v in inputs.items()}
    hidden = inp["hidden_states"].reshape(S, H).astype(np.float32)
    pos = inp["position_ids"].reshape(S).astype(np.int64)
    cosT = inp["cos"][pos].T.astype(np.float32)
    sinT = inp["sin"][pos].T.astype(np.float32)
    wq_a = (inp["wq_a"] * inp["in_ln"][:, None]).astype(np.float32)
    wkv_a = (inp["wkv_a"] * inp["in_ln"][:, None]).astype(np.float32)
    wq_b = (inp["wq_b"] * inp["q_a_ln"][:, None]).astype(np.float32)
    wkv_b = (inp["wkv_b"] * inp["kv_a_ln"][:, None]).astype(np.float32)
    wg = (inp["w_gate"] * inp["post_ln"][:, None]).astype(np.float32)
    wu = (inp["w_up"] * inp["post_ln"][:, None]).astype(np.float32)
    wd = inp["w_down"].astype(np.float32)
    wo = inp["wo"].astype(np.float32)

    de = np.empty(ROPE, np.int64)
    de[:32] = np.arange(32) * 2
    de[32:] = np.arange(32) * 2 + 1
    wkv_a = np.concatenate([wkv_a[:, :KVLR], wkv_a[:, KVLR:][:, de]], axis=1)
    wq_b = wq_b.reshape(QLR, NH, QHD)
    wkv_b = wkv_b.reshape(KVLR, NH, NOPE + VHD)

    hT = hidden.T.copy()
    sin_sg = np.concatenate([-sinT[:32], sinT[32:]], axis=0)    # signed for swap trick
    cossin = np.concatenate([cosT, cosT, sin_sg, sin_sg], axis=0)  # (256, S)
    ki = np.arange(P)[:, None]
    qi = np.arange(512)[None, :]
    dmask = np.stack([np.where(qi >= j * P + ki, 0.0, -1e30).astype(np.float32)
                      for j in range(4)], axis=1)               # (128, 4, 512)

    wq_a_t = _tile_w(wq_a)
    wkv_a_t = _tile_w(wkv_a)

    in_maps = []
    for c in range(NC):
        h0, h1 = 2 * c, 2 * c + 1
        qb = np.concatenate([
            wq_b[:, h0, :NOPE], wq_b[:, h1, :NOPE],
            wq_b[:, h0, NOPE:][:, de], wq_b[:, h1, NOPE:][:, de]], axis=1) * SCALE
        kb = np.concatenate([
            wkv_b[:, h0, :NOPE], wkv_b[:, h1, :NOPE],
            wkv_b[:, h0, NOPE:], wkv_b[:, h1, NOPE:]], axis=1)
        ssl = slice(c * SS, (c + 1) * SS)
        cs_sh = np.concatenate([cosT[:, ssl], sin_sg[:, ssl]], axis=0)
        in_maps.append({
            "hT_s": np.ascontiguousarray(hT[:, ssl]),
            "hT_r": np.ascontiguousarray(hT[ssl, :]),
            "wq_a_t": np.ascontiguousarray(
                wq_a_t.transpose(1, 2, 0, 3)).astype(bf16),
            "wkv_a_t": np.ascontiguousarray(
                wkv_a_t.transpose(1, 2, 0, 3)).astype(bf16),
            "wq_b_t": np.ascontiguousarray(_tile_w(
                qb.astype(np.float32)).transpose(2, 0, 1, 3)).astype(bf16),
            "wkv_b_t": np.ascontiguousarray(_tile_w(
                kb.astype(np.float32)).transpose(2, 0, 1, 3)).astype(bf16),
            "wo_t": np.ascontiguousarray(_tile_w(np.ascontiguousarray(
                wo[:, ssl])).transpose(2, 0, 1, 3)).astype(bf16),
            "wg_t": np.ascontiguousarray(_tile_w(
                wg[:, c * FFS:(c + 1) * FFS]).transpose(1, 2, 0, 3)).astype(bf16),
            "wu_t": np.ascontiguousarray(_tile_w(
                wu[:, c * FFS:(c + 1) * FFS]).transpose(1, 2, 0, 3)).astype(bf16),
            "wd_t": np.ascontiguousarray(_tile_w(
                wd[c * FFS:(c + 1) * FFS, :]).transpose(2, 0, 1, 3)).astype(bf16),
            "cossin": cossin,
            "cs_sh": np.ascontiguousarray(cs_sh),
            "dmask": dmask,
        })
    return in_maps


_LAST_RESULT = {}


def kernel(**inputs) -> np.ndarray:
    from concourse.bass_utils import run_bass_kernel_spmd
    nc = _build()
    in_maps = _host_prep(inputs)
    kwargs = {}
    if TRACE:
        import sys, types
        if "antenv.axon_hooks" not in sys.modules:
            try:
                from trn_agent_boot.trn_boot import _ntff_profile_via_ctypes
                mod = types.ModuleType("antenv.axon_hooks")
                _hook = _ntff_profile_via_ctypes('/opt/axon/libaxon_pjrt.so')
                mod.get_axon_ntff_profile_hook = lambda: _hook
                mod.set_axon_ntff_profile_hook = lambda h: None
                sys.modules["antenv.axon_hooks"] = mod
                import antenv
                antenv.axon_hooks = mod
            except Exception:
                pass
        kwargs["trace"] = True
    res = run_bass_kernel_spmd(nc, in_maps, list(range(NC)), **kwargs)
    _LAST_RESULT["res"] = res
    outT = np.concatenate([res.results[c]["outT"] for c in range(NC)], axis=0)
    return np.ascontiguousarray(outT.T)[None].astype(np.float32)


# revision 14
# speedup vs baseline: 1.1156x; 1.0216x over previous
"""DeepseekV3 decoder layer on 8 Trainium2 NeuronCores (Bass/Tile).

Sharding: sequence-parallel low-rank projections (one AllGather per latent
group), tensor-parallel heads for q_b/kv_b/attention (2 heads/core,
transposed-score layout), AllGather of head outputs, output-feature-sharded
o_proj + residual, AllGather of the raw post-attention hidden (post-LN stats
are recomputed locally on every core - no stats AllReduce), FF-sharded MLP
with per-chunk ReduceScatter.

Schedule: attention query chunks run in descending size order (3,2,1,0) so
the smallest chunk is last; q_b/rope for chunk j-1 and o_proj/AG3 for chunk
j+1 are interleaved between attention chunks; the MLP consumes AG3 chunks in
arrival order and the final down-proj/ReduceScatter is split 2x256 to shrink
the tail.

RMS scale-invariance: RMS() of a row of (x @ W) is independent of a
per-token scale on x, so the q/kv latent projections run directly on raw
bf16 x with no input RMS on the critical path; only k_pe (64 rows) needs
the 1/rms(x) factor. RMS/ln weights and the rope de-interleave are folded
into the weights host-side.
"""

import numpy as np

B, S, H = 1, 2048, 2048
NH, NOPE, ROPE, VHD = 16, 128, 64, 128
QHD = NOPE + ROPE
QLR, KVLR, FF = 1536, 512, 8192
SCALE = QHD ** -0.5
EPS = 1e-6
NC = 8
SS = S // NC            # 256: sequence / output-feature shard
FFS = FF // NC          # 1024: FF shard
P = 128

TRACE = False           # test.py sets kernel.TRACE = True for profiling

_CACHE = {}


def _tile_w(w):
    """[K, M] -> [K/128, ceil(M/128), 128, 128] contiguous blocks (zero-pad M)."""
    K, M = w.shape
    mc = -(-M // P)
    out = np.zeros((K // P, mc, P, P), np.float32)
    wp = np.zeros((K, mc * P), np.float32)
    wp[:, :M] = w
    for kt in range(K // P):
        for m in range(mc):
            out[kt, m] = wp[kt * P:(kt + 1) * P, m * P:(m + 1) * P]
    return out


def _build():
    if "nc" in _CACHE:
        return _CACHE["nc"]
    import concourse.mybir as mybir
    import concourse.tile as tile
    from concourse import bacc

    F32 = mybir.dt.float32
    F32R = mybir.dt.float32r
    BF16 = mybir.dt.bfloat16
    AF = mybir.ActivationFunctionType

    nc = bacc.Bacc("TRN2", target_bir_lowering=False, debug=False, num_devices=NC)

    def inp(name, shape, dt=F32):
        return nc.dram_tensor(name, list(shape), dt, kind="ExternalInput").ap()

    hT_s = inp("hT_s", [H, SS])
    hT_r = inp("hT_r", [SS, S])
    wq_a_t = inp("wq_a_t", [12, P, 16, P], BF16)
    wkv_a_t = inp("wkv_a_t", [5, P, 16, P], BF16)
    wq_b_t = inp("wq_b_t", [P, 12, 3, P], BF16)
    wkv_b_t = inp("wkv_b_t", [P, 4, 4, P], BF16)
    wo_t = inp("wo_t", [P, 16, 2, P], BF16)
    wg_t = inp("wg_t", [8, P, 16, P], BF16)
    wu_t = inp("wu_t", [8, P, 16, P], BF16)
    wd_t = inp("wd_t", [P, 8, 16, P], BF16)
    cossin = inp("cossin", [2 * P, S])        # rows 0:128 [cosT;cosT], 128:256 [sinT;sinT]
    cs_sh = inp("cs_sh", [P, SS])             # rows 0:64 cosT, 64:128 signed sinT (own shard)
    dmask = inp("dmask", [P, 4, 512])
    outT = nc.dram_tensor("outT", [SS, S], F32, kind="ExternalOutput").ap()

    RG = [list(range(NC))]

    from contextlib import ExitStack
    with tile.TileContext(nc) as tc, ExitStack() as _stack:
        cpool = _stack.enter_context(tc.tile_pool(name="const", bufs=1))
        dpool = _stack.enter_context(tc.tile_pool(name="dram", bufs=1, space="DRAM"))
        perm = _stack.enter_context(tc.tile_pool(name="perm", bufs=1))

        ag1a_in = dpool.tile([P, 5 * SS], BF16)
        ag1a_out = dpool.tile([NC * P, 5 * SS], BF16, addr_space="Shared")
        ag1b_in = dpool.tile([P, 12 * SS], BF16)
        ag1b_out = dpool.tile([NC * P, 12 * SS], BF16, addr_space="Shared")
        _ag2w = [1024, 512, 512]          # h1 (chunks 3+2), c1, c0
        ag2_in = [dpool.tile([2 * VHD, w], BF16, name=f"ag2_in{j}")
                  for j, w in enumerate(_ag2w)]
        ag2_out = [dpool.tile([NH * VHD, w], BF16, addr_space="Shared",
                              name=f"ag2_out{j}") for j, w in enumerate(_ag2w)]
        # per attn chunk j: (ag2 buffer index, col offset, oT col offset)
        AG2_MAP = {3: (0, 512, 1024), 2: (0, 0, 1024), 1: (1, 0, 512),
                   0: (2, 0, 0)}
        ag3_in = [dpool.tile([SS, 1024], BF16, name=f"ag3_in{j}") for j in range(2)]
        ag3_out = [dpool.tile([H, 1024], BF16, addr_space="Shared",
                              name=f"ag3_out{j}") for j in range(2)]
        # rs chunks: (outT column offset, width), in processing order
        RS_CH = [(1536, 512), (1024, 512), (512, 512), (0, 256), (256, 256)]
        rs_in = [dpool.tile([H, w], BF16, name=f"rs_in{j}")
                 for j, (c0, w) in enumerate(RS_CH)]
        rs_out = [dpool.tile([SS, w], BF16, name=f"rs_out{j}")
                  for j, (c0, w) in enumerate(RS_CH)]

        ones_f = cpool.tile([P, 1], F32)
        nc.vector.memset(ones_f[:], 1.0)
        ones_r = cpool.tile([P, 1], BF16)
        nc.vector.tensor_copy(ones_r[:], ones_f[:])
        eps_t = cpool.tile([P, 1], F32)
        nc.vector.memset(eps_t[:], EPS)
        ones_k1f = cpool.tile([1, P], F32)
        nc.vector.memset(ones_k1f[:], 1.0)
        ones_k1 = cpool.tile([1, P], F32R)
        nc.vector.tensor_copy(ones_k1[:], ones_k1f[:])

        h2 = perm.tile([P, 2, S], F32)        # post-attn hidden, own feature shard

        # shared psum pool for o_proj / stats / down accumulators (phases B+D)
        pbx = _stack.enter_context(tc.tile_pool(name="pbx", bufs=2, space="PSUM"))

        # ================= Stage A: seq-shard low-rank path =================
        with tc.tile_pool(name="sa", bufs=1) as sa, \
             tc.tile_pool(name="saw", bufs=5) as saw, \
             tc.tile_pool(name="pap", bufs=2, space="PSUM") as pa:
            with nc.named_scope("stageA"):
                xs = sa.tile([P, 16, SS], F32)
                nc.sync.dma_start(xs[:], hT_s.rearrange("(kt p) s -> p kt s", p=P))
                xb = sa.tile([P, 16, SS], BF16)
                for g in range(4):
                    nc.vector.tensor_copy(xb[:, 4 * g:4 * g + 4],
                                          xs[:, 4 * g:4 * g + 4])

                # kv latents on raw x (RMS scale-invariance)
                cvs = sa.tile([P, 5, SS], F32)
                for mc in range(5):
                    wt = saw.tile([P, 16, P], BF16, tag="aw")
                    nc.sync.dma_start(wt[:], wkv_a_t[mc])
                    ps = pa.tile([P, SS], F32, tag="amm")
                    for kt in range(16):
                        nc.tensor.matmul(ps[:], wt[:, kt], xb[:, kt],
                                         start=(kt == 0), stop=(kt == 15))
                    nc.vector.tensor_copy(cvs[:, mc], ps[:])

                # kv_a RMS (on raw latents; the 1/rms(x) factor cancels)
                sq3 = sa.tile([P, 4, SS], BF16)
                nc.vector.tensor_mul(sq3[:], cvs[:, :4], cvs[:, :4])
                msq3 = pa.tile([1, SS], F32, tag="acc", bufs=1)
                for mc in range(4):
                    nc.tensor.matmul(msq3[:], ones_r[:], sq3[:, mc],
                                     start=(mc == 0), stop=(mc == 3))
                r3s = sa.tile([1, SS], F32)
                nc.scalar.activation(r3s[:], msq3[:], AF.Sqrt, scale=1.0 / KVLR, bias=eps_t[:1])
                r3 = sa.tile([1, SS], F32R)
                with nc.allow_low_precision(reason="f32r rounding of rms scale"):
                    nc.vector.reciprocal(r3[:], r3s[:])
                r3bp = pa.tile([P, SS], F32, tag="rb", bufs=1)
                nc.tensor.matmul(r3bp[:], ones_k1[:], r3[:], start=True, stop=True)
                r3b = sa.tile([P, SS], F32)
                nc.vector.tensor_copy(r3b[:], r3bp[:])
                ckn = sa.tile([P, 4, SS], BF16)
                nc.vector.tensor_mul(ckn[:], cvs[:, :4],
                                     r3b[:, None, :].to_broadcast([P, 4, SS]))

                # rms(x) for the k_pe rows only
                sqx = sa.tile([P, 16, SS], BF16)
                nc.vector.tensor_mul(sqx[:], xb[:], xb[:])
                msq1 = pa.tile([1, SS], F32, tag="acc", bufs=1)
                for kt in range(16):
                    nc.tensor.matmul(msq1[:], ones_r[:], sqx[:, kt],
                                     start=(kt == 0), stop=(kt == 15))
                r1s = sa.tile([1, SS], F32)
                nc.scalar.activation(r1s[:], msq1[:], AF.Sqrt, scale=1.0 / H, bias=eps_t[:1])
                r1 = sa.tile([1, SS], F32R)
                with nc.allow_low_precision(reason="f32r rounding of rms scale"):
                    nc.vector.reciprocal(r1[:], r1s[:])
                r1bp = pa.tile([64, SS], F32, tag="rb", bufs=1)
                nc.tensor.matmul(r1bp[:], ones_k1[:, :64], r1[:], start=True, stop=True)
                r1b = sa.tile([64, SS], F32)
                nc.vector.tensor_copy(r1b[:], r1bp[:])

                # k_pe rope on cvs[:64, 4] (cs_sh rows 0:64 cos, 64:128 signed sin)
                cos_sh = sa.tile([64, SS], F32)
                nc.sync.dma_start(cos_sh[:], cs_sh[0:64, :])
                sin_sh = sa.tile([64, SS], F32)
                nc.sync.dma_start(sin_sh[:], cs_sh[64:128, :])
                ksw = sa.tile([64, SS], F32)
                nc.sync.dma_start(ksw[0:32, :], cvs[32:64, 4])
                nc.sync.dma_start(ksw[32:64, :], cvs[0:32, 4])
                kro = sa.tile([64, SS], F32)
                nc.vector.tensor_mul(kro[:], cvs[:64, 4], cos_sh[:])
                t1 = sa.tile([64, SS], F32)
                nc.vector.tensor_mul(t1[:], ksw[:], sin_sh[:])
                nc.vector.tensor_add(kro[:], kro[:], t1[:])
                kpe_n = sa.tile([64, SS], BF16)
                nc.vector.tensor_mul(kpe_n[:], kro[:], r1b[:])

                nc.sync.dma_start(
                    ag1a_in[:, 0:4 * SS].rearrange("p (kt s) -> p kt s", s=SS),
                    ckn[:])
                nc.sync.dma_start(ag1a_in[:64, 4 * SS:5 * SS], kpe_n[:])
                nc.gpsimd.collective_compute(
                    "AllGather", mybir.AluOpType.bypass, replica_groups=RG,
                    ins=[ag1a_in], outs=[ag1a_out])

                # q latents on raw x
                us = sa.tile([P, 12, SS], F32)
                for mc in range(12):
                    wt = saw.tile([P, 16, P], BF16, tag="aw")
                    nc.sync.dma_start(wt[:], wq_a_t[mc])
                    ps = pa.tile([P, SS], F32, tag="amm")
                    for kt in range(16):
                        nc.tensor.matmul(ps[:], wt[:, kt], xb[:, kt],
                                         start=(kt == 0), stop=(kt == 15))
                    nc.vector.tensor_copy(us[:, mc], ps[:])

                sq2 = sa.tile([P, 12, SS], BF16)
                nc.vector.tensor_mul(sq2[:], us[:], us[:])
                msq2 = pa.tile([1, SS], F32, tag="acc", bufs=1)
                for mc in range(12):
                    nc.tensor.matmul(msq2[:], ones_r[:], sq2[:, mc],
                                     start=(mc == 0), stop=(mc == 11))
                r2s = sa.tile([1, SS], F32)
                nc.scalar.activation(r2s[:], msq2[:], AF.Sqrt, scale=1.0 / QLR, bias=eps_t[:1])
                r2 = sa.tile([1, SS], F32R)
                with nc.allow_low_precision(reason="f32r rounding of rms scale"):
                    nc.vector.reciprocal(r2[:], r2s[:])
                r2bp = pa.tile([P, SS], F32, tag="rb", bufs=1)
                nc.tensor.matmul(r2bp[:], ones_k1[:], r2[:], start=True, stop=True)
                r2b = sa.tile([P, SS], F32)
                nc.vector.tensor_copy(r2b[:], r2bp[:])
                un = sa.tile([P, 12, SS], BF16)
                nc.vector.tensor_mul(un[:], us[:],
                                     r2b[:, None, :].to_broadcast([P, 12, SS]))
                nc.sync.dma_start(
                    ag1b_in.rearrange("p (kt s) -> p kt s", s=SS), un[:])
                nc.gpsimd.collective_compute(
                    "AllGather", mybir.AluOpType.bypass, replica_groups=RG,
                    ins=[ag1b_in], outs=[ag1b_out])

        # ===== Stage B: kv_b all blocks, per-chunk q_b/rope + attention =====
        with tc.tile_pool(name="sb2", bufs=1) as sb2, \
             tc.tile_pool(name="sbr", bufs=1) as sbr, \
             tc.tile_pool(name="sbe", bufs=1) as sbe, \
             tc.tile_pool(name="scr", bufs=2) as scr:
            kT = sb2.tile([P, 2, S], BF16)
            kpeT = sb2.tile([64, S], BF16)
            v_tok = sb2.tile([P, 2, 16, P], BF16)
            qT = sb2.tile([P, 2, S], BF16)
            qpe2 = sb2.tile([64, 2, S], BF16)
            oT = sb2.tile([P, 2, S], BF16)
            wkb = sb2.tile([P, 4, 4, P], BF16)
            wqb = sb2.tile([P, 12, 3, P], BF16)
            wos = sb2.tile([P, 16, 2, P], BF16)
            cos_t = sb2.tile([P, S], F32)
            sin_t = sb2.tile([P, S], F32)
            mask_t = sb2.tile([P, 4, 512], F32)
            nc.sync.dma_start(wkb[:], wkv_b_t[:])
            nc.sync.dma_start(wqb[:], wq_b_t[:])
            nc.sync.dma_start(cos_t[:], cossin[0:P, :])
            nc.sync.dma_start(sin_t[:], cossin[P:2 * P, :])
            nc.sync.dma_start(mask_t[:], dmask[:, :, :])
            nc.sync.dma_start(wos[:], wo_t[:])

            def oproj(j):
                """o_proj chunk j + residual -> h2, bf16 h2 into ag3_in half."""
                hf = j // 2
                off = j * 512 - hf * 1024
                bi, boff, _ = AG2_MAP[j]
                nsl = slice(j * 512, (j + 1) * 512)
                rhs = scr.tile([P, 16, 512], BF16, tag="rhs2")
                nc.sync.dma_start(
                    rhs[:],
                    ag2_out[bi].rearrange("(kt p) s -> p kt s", p=P)[
                        :, :, boff:boff + 512])
                resid = scr.tile([P, 2, 512], F32, tag="resid", bufs=1)
                nc.sync.dma_start(
                    resid[:],
                    hT_r.rearrange("(mc p) s -> p mc s", p=P)[:, :, nsl])
                h2b = scr.tile([P, 2, 512], BF16, tag="h2b")
                for mc in range(2):
                    ps = pbx.tile([P, 512], F32, tag="big")
                    for kt in range(16):
                        nc.tensor.matmul(ps[:], wos[:, kt, mc], rhs[:, kt],
                                         start=(kt == 0), stop=(kt == 15))
                    nc.vector.tensor_add(h2[:, mc, nsl], ps[:], resid[:, mc])
                    nc.vector.tensor_copy(h2b[:, mc], h2[:, mc, nsl])
                nc.sync.dma_start(
                    ag3_in[hf].rearrange("(mc p) s -> p mc s", p=P)[
                        :, :, off:off + 512], h2b[:])

            def ag3_go(hf):
                nc.gpsimd.collective_compute(
                    "AllGather", mybir.AluOpType.bypass, replica_groups=RG,
                    ins=[ag3_in[hf]], outs=[ag3_out[hf]])

            with tc.tile_pool(name="pbq", bufs=2, space="PSUM") as pbq, \
                 tc.tile_pool(name="pbo", bufs=2, space="PSUM") as pbo:
                with nc.named_scope("stageB_kv"):
                    for pr in range(4):
                        psl = slice(pr * 512, (pr + 1) * 512)
                        rhs_c = sbr.tile([P, 4, 2, SS], BF16, tag="rhs1c", bufs=2)
                        for b in range(2):
                            blk = 2 * pr + b
                            nc.sync.dma_start(
                                rhs_c[:, :, b, :],
                                ag1a_out[blk * P:(blk + 1) * P, 0:4 * SS].rearrange(
                                    "p (kt s) -> p kt s", s=SS))
                            nc.sync.dma_start(
                                kpeT[:, blk * SS:(blk + 1) * SS],
                                ag1a_out[blk * P:blk * P + 64, 4 * SS:5 * SS])
                        # k_nope (dim-major)
                        for mc in range(2):
                            ps = pbq.tile([P, 512], F32, tag="sc")
                            for kt in range(4):
                                nc.tensor.matmul(
                                    ps[:], wkb[:, kt, mc],
                                    rhs_c[:, kt].rearrange("p b s -> p (b s)"),
                                    start=(kt == 0), stop=(kt == 3))
                            nc.vector.tensor_copy(kT[:, mc, psl], ps[:])
                        # V token-major: stationary = latent tile, moving = v-cols
                        for b in range(2):
                            for st2 in range(2):
                                stile = pr * 4 + b * 2 + st2
                                pv = pbo.tile([P, 2, P], F32, tag="o")
                                for kt in range(4):
                                    nc.tensor.matmul(
                                        pv[:].rearrange("p h v -> p (h v)"),
                                        rhs_c[:, kt, b, st2 * P:(st2 + 1) * P],
                                        wkb[:, kt, 2:4, :].rearrange("p h v -> p (h v)"),
                                        start=(kt == 0), stop=(kt == 3))
                                nc.vector.tensor_copy(v_tok[:, :, stile, :], pv[:])

                def qb_rope(pr):
                    """q_b + rope for token pair-block pr (512 tokens)."""
                    psl = slice(pr * 512, (pr + 1) * 512)
                    rhs_u = sbr.tile([P, 12, 2, SS], BF16, tag="rhs1u", bufs=2)
                    for b in range(2):
                        blk = 2 * pr + b
                        nc.sync.dma_start(
                            rhs_u[:, :, b, :],
                            ag1b_out[blk * P:(blk + 1) * P, :].rearrange(
                                "p (kt s) -> p kt s", s=SS))
                    qpe_raw = sbr.tile([P, 512], F32, tag="qpr")
                    for mc in range(3):
                        ps = pbq.tile([P, 512], F32, tag="sc")
                        for kt in range(12):
                            nc.tensor.matmul(
                                ps[:], wqb[:, kt, mc],
                                rhs_u[:, kt].rearrange("p b s -> p (b s)"),
                                start=(kt == 0), stop=(kt == 11))
                        if mc < 2:
                            nc.vector.tensor_copy(qT[:, mc, psl], ps[:])
                        else:
                            nc.vector.tensor_copy(qpe_raw[:], ps[:])
                    qsw = sbr.tile([P, 512], F32, tag="qsw")
                    for qq in range(2):
                        b0 = qq * 64
                        nc.sync.dma_start(qsw[b0:b0 + 32, :],
                                          qpe_raw[b0 + 32:b0 + 64, :])
                        nc.sync.dma_start(qsw[b0 + 32:b0 + 64, :],
                                          qpe_raw[b0:b0 + 32, :])
                    qpe_rot = sbr.tile([P, 512], BF16, tag="qro")
                    nc.vector.tensor_mul(qpe_rot[:], qpe_raw[:], cos_t[:, psl])
                    t1r = sbr.tile([P, 512], F32, tag="qt1")
                    nc.vector.tensor_mul(t1r[:], qsw[:], sin_t[:, psl])
                    nc.vector.tensor_add(qpe_rot[:], qpe_rot[:], t1r[:])
                    nc.sync.dma_start(qpe2[:, 0, psl], qpe_rot[0:64, :])
                    nc.sync.dma_start(qpe2[:, 1, psl], qpe_rot[64:128, :])

                def attn_chunk(qc):
                    qsl = slice(qc * 512, (qc + 1) * 512)
                    nkt = 4 * qc + 4
                    for h in range(2):
                        o_ps = pbo.tile([P, 512], F32, tag="o")
                        d_ps = pbx.tile([1, 512], F32, tag="acc")
                        for kt in range(nkt):
                            ksl = slice(kt * P, (kt + 1) * P)
                            sc_ps = pbq.tile([P, 512], F32, tag="sc")
                            nc.tensor.matmul(sc_ps[:], kT[:, h, ksl],
                                             qT[:, h, qsl], start=True, stop=False)
                            nc.tensor.matmul(sc_ps[:], kpeT[:, ksl],
                                             qpe2[:, h, qsl], start=False, stop=True)
                            j = kt - 4 * qc
                            if j >= 0:
                                nc.vector.tensor_add(sc_ps[:], sc_ps[:],
                                                     mask_t[:, j])
                            es = sbe.tile([P, 512], BF16, tag="es", bufs=4)
                            nc.scalar.activation(es[:], sc_ps[:], AF.Exp)
                            nc.tensor.matmul(o_ps[:], v_tok[:, h, kt], es[:],
                                             start=(kt == 0), stop=(kt == nkt - 1))
                            nc.tensor.matmul(d_ps[:], ones_r[:], es[:],
                                             start=(kt == 0), stop=(kt == nkt - 1))
                        rec = sbe.tile([1, 512], F32R, tag="rec", bufs=2)
                        with nc.allow_low_precision(
                                reason="f32r rounding of softmax denom"):
                            nc.vector.reciprocal(rec[:], d_ps[:])
                        rb_ps = pbx.tile([P, 512], F32, tag="big")
                        nc.tensor.matmul(rb_ps[:], ones_k1[:], rec[:],
                                         start=True, stop=True)
                        recb = sbe.tile([P, 512], F32, tag="recb", bufs=2)
                        nc.vector.tensor_copy(recb[:], rb_ps[:])
                        nc.vector.tensor_mul(oT[:, h, qsl], o_ps[:], recb[:])

                def ag2_go(bi, o_off, w):
                    nc.sync.dma_start(
                        ag2_in[bi].rearrange("(mc p) s -> p mc s", p=P),
                        oT[:, :, o_off:o_off + w])
                    nc.gpsimd.collective_compute(
                        "AllGather", mybir.AluOpType.bypass, replica_groups=RG,
                        ins=[ag2_in[bi]], outs=[ag2_out[bi]])

                with nc.named_scope("stageB_attn"):
                    qb_rope(3)
                    qb_rope(2)
                    attn_chunk(3)
                    qb_rope(1)
                    attn_chunk(2)
                    ag2_go(0, 1024, 1024)
                    qb_rope(0)
                    attn_chunk(1)
                    ag2_go(1, 512, 512)
                    oproj(3)
                    oproj(2)
                    attn_chunk(0)
                    ag3_go(1)
                    ag2_go(2, 0, 512)
                    oproj(1)
                    oproj(0)
                    ag3_go(0)

        # ================= Stage D: post-LN + MLP, chunk pipelined =============
        with tc.tile_pool(name="wmlp", bufs=1) as wmlp, \
             tc.tile_pool(name="wstr", bufs=1) as wstr, \
             tc.tile_pool(name="smy", bufs=1) as smy, \
             tc.tile_pool(name="sdd", bufs=2) as sdd, \
             tc.tile_pool(name="pgu", bufs=1, space="PSUM") as pgu:
            with nc.named_scope("stageD"):
                wds = wmlp.tile([P, 8, 16, P], BF16)
                nc.sync.dma_start(wds[:], wd_t[:])

                def hy_fetch(j):
                    hf = j // 2
                    off = j * 512 - hf * 1024
                    hy = smy.tile([P, 16, 512], BF16, tag="hy", bufs=3)
                    nc.sync.dma_start(
                        hy[:],
                        ag3_out[hf].rearrange("(kt p) s -> p kt s", p=P)[
                            :, :, off:off + 512])
                    return hy

                def stats(hy):
                    """Local post-LN stats for a chunk -> r4b broadcast tile.
                    The 1/rms scale is applied at gate/up/down psum readout
                    (it commutes through the linear layers)."""
                    m4 = pbx.tile([1, 512], F32, tag="acc")
                    for half in range(2):
                        sqh = smy.tile([P, 8, 512], BF16, tag="sqh", bufs=1)
                        nc.vector.tensor_mul(sqh[:], hy[:, half * 8:half * 8 + 8],
                                             hy[:, half * 8:half * 8 + 8])
                        for kt in range(8):
                            nc.tensor.matmul(m4[:], ones_r[:], sqh[:, kt],
                                             start=(half == 0 and kt == 0),
                                             stop=(half == 1 and kt == 7))
                    r4s = smy.tile([1, 512], F32, tag="r4s", bufs=2)
                    nc.scalar.activation(r4s[:], m4[:], AF.Sqrt,
                                         scale=1.0 / H, bias=eps_t[:1])
                    r4 = smy.tile([1, 512], F32R, tag="r4", bufs=2)
                    with nc.allow_low_precision(reason="f32r rounding of rms scale"):
                        nc.vector.reciprocal(r4[:], r4s[:])
                    r4bp = pbx.tile([P, 512], F32, tag="big")
                    nc.tensor.matmul(r4bp[:], ones_k1[:], r4[:],
                                     start=True, stop=True)
                    r4b = smy.tile([P, 512], F32, tag="r4b", bufs=2)
                    nc.vector.tensor_copy(r4b[:], r4bp[:])
                    return r4b

                def gateup(y, r4b, y_off, cw):
                    """gate/up on raw y[:, :, y_off:y_off+cw]; r4 scale applied
                    to the silu input (the up-branch scale rides to down)."""
                    act = smy.tile([P, 8, 512], BF16, tag="act", bufs=2,
                                   name="act")[:, :, :cw]
                    for m in range(8):
                        wgm = wstr.tile([P, 16, P], BF16, tag="wg", bufs=4)
                        nc.sync.dma_start(wgm[:], wg_t[m])
                        wum = wstr.tile([P, 16, P], BF16, tag="wu", bufs=4)
                        nc.sync.dma_start(wum[:], wu_t[m])
                        gp = pgu.tile([P, 512], F32, tag=f"g{m % 2}",
                                      name="gp")[:, :cw]
                        up = pgu.tile([P, 512], F32, tag=f"u{m % 2}",
                                      name="up")[:, :cw]
                        for kt in range(16):
                            nc.tensor.matmul(gp[:], wgm[:, kt],
                                             y[:, kt, y_off:y_off + cw],
                                             start=(kt == 0), stop=(kt == 15))
                            nc.tensor.matmul(up[:], wum[:, kt],
                                             y[:, kt, y_off:y_off + cw],
                                             start=(kt == 0), stop=(kt == 15))
                        t1 = smy.tile([P, 512], F32, tag="t1", bufs=2,
                                      name="t1")[:, :cw]
                        nc.vector.tensor_mul(t1[:], gp[:],
                                             r4b[:, y_off:y_off + cw])
                        gsil = sdd.tile([P, 512], BF16, tag="gsil",
                                        name="gsil")[:, :cw]
                        nc.scalar.activation(gsil[:], t1[:], AF.Silu)
                        nc.vector.tensor_mul(act[:, m], gsil[:], up[:])
                    return act

                def down(act, r4b, act_off, ri, r4_off=None):
                    """down-proj of act[:, :, act_off:act_off+w] -> RS chunk ri;
                    applies the deferred up-branch r4 scale at psum readout."""
                    c0, cw = RS_CH[ri]
                    if r4_off is None:
                        r4_off = act_off
                    nsl = slice(c0, c0 + cw)
                    for q in range(4):
                        dn = sdd.tile([P, 4, 512], BF16, tag="dn",
                                      name="dn")[:, :, :cw]
                        for s in range(4):
                            mc = 4 * q + s
                            ps = pbx.tile([P, 512], F32, tag="big",
                                          name="dps")[:, :cw]
                            for kt in range(8):
                                nc.tensor.matmul(
                                    ps[:], wds[:, kt, mc],
                                    act[:, kt, act_off:act_off + cw],
                                    start=(kt == 0), stop=(kt == 7))
                            nc.vector.tensor_mul(
                                dn[:, s], ps[:], r4b[:, r4_off:r4_off + cw])
                        nc.sync.dma_start(
                            rs_in[ri][4 * q * P:4 * (q + 1) * P, :].rearrange(
                                "(q p) s -> p q s", p=P), dn[:])
                    nc.gpsimd.collective_compute(
                        "ReduceScatter", mybir.AluOpType.add, replica_groups=RG,
                        ins=[rs_in[ri]], outs=[rs_out[ri]])
                    fin = sdd.tile([P, 2, 512], BF16, tag="fin",
                                   name="fin")[:, :, :cw]
                    nc.sync.dma_start(
                        fin[:], rs_out[ri].rearrange("(mc p) s -> p mc s", p=P))
                    fino = sdd.tile([P, 2, 512], F32, tag="fino", name="fino",
                                    bufs=1)[:, :, :cw]
                    nc.vector.tensor_add(fino[:], fin[:], h2[:, :, nsl])
                    nc.sync.dma_start(
                        outT.rearrange("(mc p) s -> p mc s", p=P)[:, :, nsl],
                        fino[:])

                hy3 = hy_fetch(3)
                r43 = stats(hy3)
                hy2 = hy_fetch(2)
                act3 = gateup(hy3, r43, 0, 512)
                r42 = stats(hy2)
                down(act3, r43, 0, 0)
                hy1 = hy_fetch(1)
                act2 = gateup(hy2, r42, 0, 512)
                r41 = stats(hy1)
                down(act2, r42, 0, 1)
                hy0 = hy_fetch(0)
                act1 = gateup(hy1, r41, 0, 512)
                r40 = stats(hy0)
                down(act1, r41, 0, 2)
                act0a = gateup(hy0, r40, 0, 256)
                down(act0a, r40, 0, 3)
                act0b = gateup(hy0, r40, 256, 256)
                down(act0b, r40, 0, 4, r4_off=256)

    nc.compile()
    _CACHE["nc"] = nc
    return nc


def _host_prep(inputs):
    import ml_dtypes
    bf16 = ml_dtypes.bfloat16
    inp = {k: np.asarray(v) for k, v in inputs.items()}
    hidden = inp["hidden_states"].reshape(S, H).astype(np.float32)
    pos = inp["position_ids"].reshape(S).astype(np.int64)
    cosT = inp["cos"][pos].T.astype(np.float32)
    sinT = inp["sin"][pos].T.astype(np.float32)
    wq_a = (inp["wq_a"] * inp["in_ln"][:, None]).astype(np.float32)
    wkv_a = (inp["wkv_a"] * inp["in_ln"][:, None]).astype(np.float32)
    wq_b = (inp["wq_b"] * inp["q_a_ln"][:, None]).astype(np.float32)
    wkv_b = (inp["wkv_b"] * inp["kv_a_ln"][:, None]).astype(np.float32)
    wg = (inp["w_gate"] * inp["post_ln"][:, None]).astype(np.float32)
    wu = (inp["w_up"] * inp["post_ln"][:, None]).astype(np.float32)
    wd = inp["w_down"].astype(np.float32)
    wo = inp["wo"].astype(np.float32)

    de = np.empty(ROPE, np.int64)
    de[:32] = np.arange(32) * 2
    de[32:] = np.arange(32) * 2 + 1
    wkv_a = np.concatenate([wkv_a[:, :KVLR], wkv_a[:, KVLR:][:, de]], axis=1)
    wq_b = wq_b.reshape(QLR, NH, QHD)
    wkv_b = wkv_b.reshape(KVLR, NH, NOPE + VHD)

    hT = hidden.T.copy()
    sin_sg = np.concatenate([-sinT[:32], sinT[32:]], axis=0)    # signed for swap trick
    cossin = np.concatenate([cosT, cosT, sin_sg, sin_sg], axis=0)  # (256, S)
    ki = np.arange(P)[:, None]
    qi = np.arange(512)[None, :]
    dmask = np.stack([np.where(qi >= j * P + ki, 0.0, -1e30).astype(np.float32)
                      for j in range(4)], axis=1)               # (128, 4, 512)

    wq_a_t = _tile_w(wq_a)
    wkv_a_t = _tile_w(wkv_a)

    in_maps = []
    for c in range(NC):
        h0, h1 = 2 * c, 2 * c + 1
        qb = np.concatenate([
            wq_b[:, h0, :NOPE], wq_b[:, h1, :NOPE],
            wq_b[:, h0, NOPE:][:, de], wq_b[:, h1, NOPE:][:, de]], axis=1) * SCALE
        kb = np.concatenate([
            wkv_b[:, h0, :NOPE], wkv_b[:, h1, :NOPE],
            wkv_b[:, h0, NOPE:], wkv_b[:, h1, NOPE:]], axis=1)
        ssl = slice(c * SS, (c + 1) * SS)
        cs_sh = np.concatenate([cosT[:, ssl], sin_sg[:, ssl]], axis=0)
        in_maps.append({
            "hT_s": np.ascontiguousarray(hT[:, ssl]),
            "hT_r": np.ascontiguousarray(hT[ssl, :]),
            "wq_a_t": np.ascontiguousarray(
                wq_a_t.transpose(1, 2, 0, 3)).astype(bf16),
            "wkv_a_t": np.ascontiguousarray(
                wkv_a_t.transpose(1, 2, 0, 3)).astype(bf16),
            "wq_b_t": np.ascontiguousarray(_tile_w(
                qb.astype(np.float32)).transpose(2, 0, 1, 3)).astype(bf16),
            "wkv_b_t": np.ascontiguousarray(_tile_w(
                kb.astype(np.float32)).transpose(2, 0, 1, 3)).astype(bf16),
            "wo_t": np.ascontiguousarray(_tile_w(np.ascontiguousarray(
                wo[:, ssl])).transpose(2, 0, 1, 3)).astype(bf16),
            "wg_t": np.ascontiguousarray(_tile_w(
                wg[:, c * FFS:(c + 1) * FFS]).transpose(1, 2, 0, 3)).astype(bf16),
            "wu_t": np.ascontiguousarray(_tile_w(
                wu[:, c * FFS:(c + 1) * FFS]).transpose(1, 2, 0, 3)).astype(bf16),
            "wd_t": np.ascontiguousarray(_tile_w(
                wd[c * FFS:(c + 1) * FFS, :]).transpose(2, 0, 1, 3)).astype(bf16),
            "cossin": cossin,
            "cs_sh": np.ascontiguousarray(cs_sh),
            "dmask": dmask,
        })
    return in_maps


_LAST_RESULT = {}


def kernel(**inputs) -> np.ndarray:
    from concourse.bass_utils import run_bass_kernel_spmd
    nc = _build()
    in_maps = _host_prep(inputs)
    kwargs = {}
    if TRACE:
        import sys, types
        if "antenv.axon_hooks" not in sys.modules:
            try:
                from trn_agent_boot.trn_boot import _ntff_profile_via_ctypes
                mod = types.ModuleType("antenv.axon_hooks")
                _hook = _ntff_profile_via_ctypes('/opt/axon/libaxon_pjrt.so')
                mod.get_axon_ntff_profile_hook = lambda: _hook
                mod.set_axon_ntff_profile_hook = lambda h: None
                sys.modules["antenv.axon_hooks"] = mod
                import antenv
                antenv.axon_hooks = mod
            except Exception:
                pass
        kwargs["trace"] = True
    res = run_bass_kernel_spmd(nc, in_maps, list(range(NC)), **kwargs)
    _LAST_RESULT["res"] = res
    outT = np.concatenate([res.results[c]["outT"] for c in range(NC)], axis=0)
    return np.ascontiguousarray(outT.T)[None].astype(np.float32)
